# revision 23
# baseline (speedup 1.0000x reference)
"""Longformer block on 8 TRN2 NeuronCores (Bass/Tile, SPMD).

Sharding: data-parallel over (batch, sequence): core c -> batch c//4, token
chunk (c%4)*512..+512. Weights replicated (fp8 e4m3, scaled x64, with an
fp8 residual-compensation tensor for W1/W2). On-chip layout is transposed
[D, token] so LN/residual/matmuls need no device transposes (host
pre-transposes x; LN stats via ones-vector f32r matmuls on PE).

Big GEMMs run as fp8 DoubleRow matmuls (two 128-deep k-tiles per
instruction). The FFN uses residual-compensated fp8:
  FFN1: W1hi@h_hi + W1lo@h_hi + W1hi@h_lo     (3 DoubleRows per 2 k-tiles)
  FFN2: W2hi@g + W2lo@g                        (2 DoubleRows per 2 k-tiles)
which keeps the fp8 quantization error at second order.

Attention: banded causal window (halo of 128 tokens recomputed locally) +
the token-0 global column as a 257th score column. The one global *row*
(token T-1) is computed via per-core exp-sum partials over each core's own
K/V slice, combined with an in-kernel AllReduce, and patched into the owning
core's output column with copy_predicated.
"""

from collections import deque

import numpy as np
import ml_dtypes

import concourse.bass as bass
import concourse.mybir as mybir
import concourse.tile as tile
from concourse.masks import make_identity
from concourse.bass_utils import run_bass_kernel_spmd

F32 = mybir.dt.float32
F32R = mybir.dt.float32r
BF16 = mybir.dt.bfloat16
F8 = mybir.dt.float8e4
AF = mybir.ActivationFunctionType
ALU = mybir.AluOpType
AX = mybir.AxisListType
DR = mybir.MatmulPerfMode.DoubleRow

D = 1024
FF = 4096
H = 16
HD = 64
T = 2048
B = 2
CHUNK = 512
HALO = 128
NSLOT = 768          # [halo 128 | own 512 | t0 | t2047 | pad]
NKV = 641            # slots 0..640 hold K/V (640 = token0); 641 = q2047 src
NQB = 4
WIN = 256
NEG = -1e30
EPS = 1e-5
N_CORES = 8
WS = 64.0            # fp8 weight pre-scale
SKIP_CC = [False]   # set kernel.SKIP_CC[0]=True to build without the
                    # collective (TimelineSim is single-core only)
PHASE_MARKS = []    # (phase_name, first_inst_id) filled during _emit


def _mark(nc, name):
    PHASE_MARKS.append((name, set(nc.inst_map.keys())))

# ---------------------------------------------------------------- bir fix ---

_waitfix_ctr = [0]


def _split_multiwaits(nc):
    """This container's walrus accepts ONE sync-wait per instruction; Tile
    attaches several. Hoist extras onto NoOps just before each instruction
    (Tile sems are monotonic within a context, so sequential waits are
    equivalent)."""
    n = 0
    for func in nc.m.functions:
        for bb in func.blocks:
            out = []
            changed = False
            for inst in bb.instructions:
                si = inst.sync_info
                if si is not None and len(si.on_wait) > 1:
                    waits = list(si.on_wait)
                    keep = [w for w in waits
                            if getattr(w, "wait_mode", "") not in
                            ("sem-ge-imm", "sem-ge-reg")]
                    if keep:
                        hoist = [w for w in waits if w not in keep]
                        last = keep
                    else:
                        hoist, last = waits[:-1], [waits[-1]]
                    for w in hoist:
                        _waitfix_ctr[0] += 1
                        nop = mybir.InstNoOp(name=f"I-waitfix-{_waitfix_ctr[0]}")
                        nop.engine = inst.engine
                        nop.sync_info = mybir.SyncInfo(on_wait=[w], on_update=[])
                        out.append(nop)
                        n += 1
                    si.on_wait = last
                    changed = True
                out.append(inst)
            if changed:
                bb.instructions[:] = out
    return n

# ------------------------------------------------------------ host helpers --


def _make_x_ext(x, c):
    b, j = divmod(c, 4)
    start = j * CHUNK
    ext = np.zeros((NSLOT, D), np.float32)
    ext[0:HALO] = x[b, start - HALO:start] if j > 0 else x[b, 0:HALO]
    ext[HALO:HALO + CHUNK] = x[b, start:start + CHUNK]
    ext[640] = x[b, 0]
    ext[641] = x[b, T - 1]
    return ext


def _make_mask(c):
    b, j = divmod(c, 4)
    start = j * CHUNK
    m = np.full((NQB, 128, WIN + 1), NEG, np.float32)
    il = np.arange(128)[:, None]
    jl = np.arange(WIN)[None, :]
    for qb in range(NQB):
        q_abs = start + qb * 128 + il
        slot = qb * 128 + jl
        band = (jl >= il) & (jl <= il + 128)
        valid = (j > 0) | (slot >= HALO)
        blk = m[qb, :, :WIN]
        blk[band & valid] = 0.0
        tok0_in_band = (q_abs[:, 0] <= HALO) & (j == 0)
        m[qb, :, WIN] = np.where(tok0_in_band, NEG, 0.0)
    return m


def _tileP(a, p=128):
    """[N*p, ...] -> [p, N, ...] partition-tiled layout."""
    n = a.shape[0] // p
    return np.ascontiguousarray(
        a.reshape(n, p, *a.shape[1:]).transpose(1, 0, *range(2, a.ndim + 1)))


def _vec_t(v):
    return np.ascontiguousarray(np.asarray(v, np.float32).reshape(-1, 128).T)

# ------------------------------------------------------------ bass program --


def _build_nc():
    nc = bass.Bass()

    f8 = mybir.dt
    inp = {}
    for name, shape, dt in [
        ("xT", [128, 8, NSLOT], F32),
        ("wq8", [128, 8, D], F8), ("wk8", [128, 8, D], F8),
        ("wv8", [128, 8, D], F8), ("wo8", [128, 8, D], F8),
        ("w1hi", [128, 8, FF], F8), ("w1lo", [128, 8, FF], F8),
        ("w2hi", [128, 32, D], F8), ("w2lo", [128, 32, D], F8),
        ("msk", [128, NQB, WIN + 1], BF16),
        ("pvec", [128, 6, 8], F32),       # g1,b1,g2,b2,bo,b2
        ("b1h", [128, 32], F32),
        ("fixsel", [128, 1], mybir.dt.uint8),
        ("fAB", [16, 2], F32),
    ]:
        inp[name] = nc.dram_tensor(name, shape, dt, kind="ExternalInput")
    out_d = nc.dram_tensor("outT", [128, 8, CHUNK], F32, kind="ExternalOutput")
    pin = nc.dram_tensor("pin", [H, 2, HD + 1], F32)
    pout = nc.dram_tensor("pout", [H, 2, HD + 1], F32, addr_space="Shared")

    with tile.TileContext(nc) as tc:
        _emit(nc, tc, inp, out_d, pin, pout)
    _split_multiwaits(nc)
    return nc


def _emit(nc, tc, inp, out_d, pin, pout):
    from contextlib import ExitStack
    ctx = ExitStack()
    with ctx:
        pers = ctx.enter_context(tc.tile_pool(name="pers", bufs=1))
        small = ctx.enter_context(tc.tile_pool(name="small", bufs=2))
        bigY = ctx.enter_context(tc.tile_pool(name="bigY", bufs=1))
        big0 = ctx.enter_context(tc.tile_pool(name="big0", bufs=1))
        w1p = ctx.enter_context(tc.tile_pool(name="w1p", bufs=3))

        # ---- persistent constants / params
        pvec = pers.tile([128, 6, 8], F32, tag="pvec", name="pvec")
        b1h = pers.tile([128, 32], F32, tag="b1h", name="b1h")
        msk = pers.tile([128, NQB, WIN + 1], BF16, tag="msk")
        fixsel = pers.tile([128, 1], mybir.dt.uint8, tag="fixsel")
        fAB = pers.tile([16, 2], F32, tag="fAB")

        idf = pers.tile([128, 128], F32, tag="idf")
        idb = pers.tile([128, 128], BF16, tag="idb")
        onesp = pers.tile([128, 1], BF16, tag="onesp")
        epst = pers.tile([1, 1], F32, tag="epst")
        neg3 = pers.tile([128, 1], F32, tag="neg3")

        yT = bigY.tile([128, 8, CHUNK], F32, tag="yT")
        h2hi = bigY.tile([128, 8, CHUNK], F8, tag="h2hi")
        h2lo = bigY.tile([128, 8, CHUNK], F8, tag="h2lo")
        w2hi = big0.tile([128, 32, D], F8, tag="w2hi")
        w1sl = {}   # FFN1 weight slab stream: g -> (hi, lo), 4 m-tiles each

        def w1_issue(g):
            if g >= 8:
                return
            hi = w1p.tile([128, 8, 512], F8, tag="w1hi")
            lo = w1p.tile([128, 8, 512], F8, tag="w1lo")
            gs = slice(g * 512, (g + 1) * 512)
            nc.sync.dma_start(out=hi, in_=inp["w1hi"][:, :, gs])
            nc.sync.dma_start(out=lo, in_=inp["w1lo"][:, :, gs])
            w1sl[g] = (hi, lo)

        def g_(i, kt):   # per-partition scalar views of packed params
            return pvec[:, i, kt:kt + 1]

        # ================= LN in transposed layout (bf16 stats) ============
        def layernorm_T(src, chunks, emit_out, pools):
            """emit_out(kt, sl, t2): consume normalized bf16 (pre-g/b).
            chunks: list of (start, end) column ranges to process."""
            ps_row, ps_bc, scr, rowp = pools
            width = max(e for _, e in chunks)
            mus = []
            for (cs, ce) in chunks:
                mus.append((ps_row.tile([1, ce - cs], F32, tag="row",
                                        name="mu"),
                            ps_row.tile([1, ce - cs], F32, tag="row",
                                        name="msq")))
            onesb = scr.tile([128, 1], BF16, tag="ln_onesb")
            nc.vector.memset(onesb, 1.0 / D)
            ones1b = scr.tile([1, 128], BF16, tag="ln_ones1b")
            nc.vector.memset(ones1b, 1.0)
            for kt in range(8):
                xb = scr.tile([128, width], BF16, tag="ln_xb")
                nc.scalar.copy(out=xb, in_=src[:, kt, 0:width])
                xsq = scr.tile([128, width], BF16, tag="ln_xsq")
                nc.vector.tensor_mul(out=xsq, in0=xb, in1=xb)
                for cch, (cs, ce) in enumerate(chunks):
                    sl = slice(cs, ce)
                    nc.tensor.matmul(mus[cch][0], onesb, xb[:, sl],
                                     start=kt == 0, stop=kt == 7)
                    nc.tensor.matmul(mus[cch][1], onesb, xsq[:, sl],
                                     start=kt == 0, stop=kt == 7)
            bcs = []
            for cch, (cs, ce) in enumerate(chunks):
                cw = ce - cs
                mu_ps, msq_ps = mus[cch]
                musb = rowp.tile([1, cw], F32, tag="ln_mu")
                nc.scalar.copy(out=musb, in_=mu_ps)
                tmp = rowp.tile([1, cw], F32, tag="ln_tmp")
                nc.vector.tensor_mul(out=tmp, in0=musb, in1=musb)
                nc.vector.tensor_sub(out=tmp, in0=msq_ps, in1=tmp)
                nc.scalar.activation(out=tmp, in_=tmp, func=AF.Sqrt,
                                     bias=epst, scale=1.0)
                nc.vector.reciprocal(out=tmp, in_=tmp)       # rstd
                tmpb = rowp.tile([1, cw], BF16, tag="ln_tmpb")
                nc.vector.tensor_copy(out=tmpb, in_=tmp)
                nc.vector.tensor_mul(out=musb, in0=musb, in1=tmp)
                musbb = rowp.tile([1, cw], BF16, tag="ln_musbb")
                nc.scalar.mul(out=musbb, in_=musb, mul=-1.0)  # -mu*rstd
                rb_ps = ps_bc.tile([128, cw], F32, tag="bc", name="rb")
                nc.tensor.matmul(rb_ps, ones1b, tmpb, start=True, stop=True)
                nb_ps = ps_bc.tile([128, cw], F32, tag="bc", name="nb")
                nc.tensor.matmul(nb_ps, ones1b, musbb, start=True, stop=True)
                rb_sb = scr.tile([128, cw], BF16, tag="ln_rb")
                nc.scalar.copy(out=rb_sb, in_=rb_ps)
                nb_sb = scr.tile([128, cw], BF16, tag="ln_nb")
                nc.scalar.copy(out=nb_sb, in_=nb_ps)
                bcs.append((rb_sb, nb_sb))
            for kt in range(8):
                for cch, (cs, ce) in enumerate(chunks):
                    sl = slice(cs, ce)
                    rb_sb, nb_sb = bcs[cch]
                    t1 = scr.tile([128, ce - cs], BF16, tag="ln_t1")
                    nc.vector.tensor_mul(out=t1, in0=src[:, kt, sl],
                                         in1=rb_sb)
                    t2 = scr.tile([128, ce - cs], BF16, tag="ln_t2")
                    nc.vector.tensor_add(out=t2, in0=t1, in1=nb_sb)
                    emit_out(kt, sl, t2)

        with tc.tile_pool(name="bigA", bufs=1) as bigA:
            # ---- long-lived activations (until end of phase G)
            xT = bigA.tile([128, 8, NSLOT], F32, tag="xT")
            nc.sync.dma_start(out=xT[:, 0, 0:384], in_=inp["xT"][:, 0, 0:384])
            nc.sync.dma_start(out=xT[:, 0, 384:NSLOT],
                              in_=inp["xT"][:, 0, 384:NSLOT])
            for kt in range(1, 8):
                nc.sync.dma_start(out=xT[:, kt, :], in_=inp["xT"][:, kt, :])
            nc.sync.dma_start(out=pvec, in_=inp["pvec"][:])
            nc.sync.dma_start(out=b1h, in_=inp["b1h"][:])
            nc.sync.dma_start(out=msk, in_=inp["msk"][:])
            nc.sync.dma_start(out=fixsel, in_=inp["fixsel"][:])
            nc.sync.dma_start(out=fAB, in_=inp["fAB"][:])
            make_identity(nc, idf)
            make_identity(nc, idb)
            nc.vector.memset(onesp, 1.0)
            nc.vector.memset(epst, EPS)
            nc.vector.memset(neg3, -3.0)

            wo8 = bigA.tile([128, 8, D], F8, tag="wo8")
            h8 = bigA.tile([128, 8, NSLOT], F8, tag="h8")
            QT = bigA.tile([128, 8, CHUNK], BF16, tag="QT")
            q47T = bigA.tile([128, 8], BF16, tag="q47T")
            KT = bigA.tile([128, 8, NKV], BF16, tag="KT")
            V = bigA.tile([128, 6, D], BF16, tag="V")
            OT = bigA.tile([128, 8, CHUNK], F8, tag="OT")

            with tc.tile_pool(name="bigW", bufs=1) as bigW:
                wq8 = bigW.tile([128, 8, D], F8, tag="wq8")
                nc.sync.dma_start(out=wq8[:, :, 0:512],
                                  in_=inp["wq8"][:, :, 0:512])
                wk8 = bigW.tile([128, 8, D], F8, tag="wk8")
                nc.sync.dma_start(out=wk8[:, :, 0:512],
                                  in_=inp["wk8"][:, :, 0:512])
                nc.sync.dma_start(out=wq8[:, :, 512:D],
                                  in_=inp["wq8"][:, :, 512:D])
                nc.sync.dma_start(out=wk8[:, :, 512:D],
                                  in_=inp["wk8"][:, :, 512:D])
                wv8 = bigW.tile([128, 8, D], F8, tag="wv8")
                nc.sync.dma_start(out=wv8, in_=inp["wv8"][:])
                nc.sync.dma_start(out=wo8, in_=inp["wo8"][:])
                for g in range(4):
                    gs = slice(g * 8, (g + 1) * 8)
                    nc.sync.dma_start(out=w2hi[:, gs, :],
                                      in_=inp["w2hi"][:, gs, :])

                _mark(nc, "B:ln1")
                # ============= Phase B: LN1 -> h8 (fp8) ====================
                with tc.tile_pool(name="ps_row1", bufs=4, space="PSUM") as ps_row, \
                     tc.tile_pool(name="ps_bc1", bufs=4, space="PSUM") as ps_bc, \
                     tc.tile_pool(name="lnrow1", bufs=1) as lnrow, \
                     tc.tile_pool(name="lnscr1", bufs=2) as lnscr:
                    def ln1_out(kt, sl, t2):
                        nc.gpsimd.tensor_scalar(
                            out=h8[:, kt, sl], in0=t2,
                            scalar1=g_(0, kt), scalar2=g_(1, kt),
                            op0=ALU.mult, op1=ALU.add)
                    layernorm_T(xT, [(0, 384), (384, 642)], ln1_out,
                                (ps_row, ps_bc, lnscr, lnrow))

                _mark(nc, "C:qkv")
                # ============= Phase C: QKV (fp8 DoubleRow) ================
                with tc.tile_pool(name="ps_big", bufs=6, space="PSUM") as ps_big, \
                     tc.tile_pool(name="ps_tiny", bufs=2, space="PSUM") as ps_tiny:
                    for m in range(8):
                        msl = slice(m * 128, (m + 1) * 128)
                        q_ps = ps_big.tile([128, CHUNK], F32, tag="big")
                        q47_ps = ps_tiny.tile([128, 1], F32, tag="tiny")
                        for t in range(0, 8, 2):
                            nc.tensor.matmul(
                                q_ps, wq8[:, t:t + 2, msl],
                                h8[:, t:t + 2, HALO:HALO + CHUNK],
                                start=t == 0, stop=t == 6, perf_mode=DR)
                            nc.tensor.matmul(
                                q47_ps, wq8[:, t, msl], h8[:, t, 641:642],
                                start=t == 0, stop=False)
                            nc.tensor.matmul(
                                q47_ps, wq8[:, t + 1, msl],
                                h8[:, t + 1, 641:642],
                                start=False, stop=t == 6)
                        nc.vector.tensor_scalar_mul(
                            out=QT[:, m, :], in0=q_ps,
                            scalar1=1.0 / (WS * np.sqrt(HD)))
                        nc.scalar.mul(out=q47T[:, m:m + 1], in_=q47_ps,
                                      mul=1.0 / (WS * np.sqrt(HD)))
                        k_ps = ps_big.tile([128, 512], F32, tag="big")
                        k_ps2 = ps_big.tile([128, NKV - 512], F32, tag="big")
                        for t in range(0, 8, 2):
                            nc.tensor.matmul(
                                k_ps, wk8[:, t:t + 2, msl],
                                h8[:, t:t + 2, 0:512],
                                start=t == 0, stop=t == 6, perf_mode=DR)
                            nc.tensor.matmul(
                                k_ps2, wk8[:, t:t + 2, msl],
                                h8[:, t:t + 2, 512:NKV],
                                start=t == 0, stop=t == 6, perf_mode=DR)
                        nc.scalar.mul(out=KT[:, m, 0:512], in_=k_ps,
                                      mul=1.0 / WS)
                        nc.vector.tensor_scalar_mul(out=KT[:, m, 512:NKV],
                                                    in0=k_ps2,
                                                    scalar1=1.0 / WS)
                    for tt in range(6):
                        for cch in range(2):
                            csl = slice(cch * 512, (cch + 1) * 512)
                            v_ps = ps_big.tile([128, 512], F32, tag="big")
                            for t in range(0, 8, 2):
                                nc.tensor.matmul(
                                    v_ps,
                                    h8[:, t:t + 2, tt * 128:(tt + 1) * 128],
                                    wv8[:, t:t + 2, csl],
                                    start=t == 0, stop=t == 6, perf_mode=DR)
                            nc.scalar.mul(out=V[:, tt, csl][:, 0:256],
                                          in_=v_ps[:, 0:256], mul=1.0 / WS)
                            nc.vector.tensor_scalar_mul(
                                out=V[:, tt, csl][:, 256:512],
                                in0=v_ps[:, 256:512], scalar1=1.0 / WS)

                    _mark(nc, "D:partials")
                    # ========= Phase D: global-row partials + AllReduce ====
                    sT = small.tile([128, H * 4], F32, tag="p_sT")
                    for h in range(H):
                        p0 = 64 * (h % 2)
                        s47_ps = ps_tiny.tile([128, 4], F32, tag="tiny")
                        for i in range(4):
                            nc.tensor.matmul(
                                s47_ps[:, i:i + 1],
                                KT[p0:p0 + 64, h // 2,
                                   HALO + 128 * i:HALO + 128 * (i + 1)],
                                q47T[p0:p0 + 64, h // 2:h // 2 + 1],
                                start=True, stop=True)
                        nc.scalar.copy(out=sT[:, 4 * h:4 * h + 4], in_=s47_ps)
                    p47 = small.tile([128, H * 4], BF16, tag="p_p47")
                    nc.scalar.activation(out=p47, in_=sT, func=AF.Exp)
                    ssum_ps = ps_tiny.tile([1, H * 4], F32, tag="tiny")
                    nc.tensor.matmul(ssum_ps, onesp, p47, start=True,
                                     stop=True)
                    s_c = small.tile([1, H], F32, tag="p_sc")
                    nc.vector.reduce_sum(
                        out=s_c,
                        in_=ssum_ps.rearrange("p (h i) -> p h i", i=4),
                        axis=AX.X)
                    oall = small.tile([65, H], F32, tag="p_oall")
                    for h in range(H):
                        o47_ps = ps_tiny.tile([64, 1], F32, tag="tiny")
                        for i in range(4):
                            nc.tensor.matmul(
                                o47_ps, V[:, 1 + i, 64 * h:64 * h + 64],
                                p47[:, 4 * h + i:4 * h + i + 1],
                                start=i == 0, stop=i == 3)
                        nc.scalar.copy(out=oall[0:64, h:h + 1], in_=o47_ps)
                    nc.sync.dma_start(out=oall[64:65, :], in_=s_c)
                    part_ps = ps_tiny.tile([H, 65], F32, tag="tiny")
                    nc.tensor.transpose(part_ps, oall, idf[0:65, 0:65])
                    part_sb = small.tile([H, 65], F32, tag="p_part")
                    nc.scalar.copy(out=part_sb, in_=part_ps)
                    pa = small.tile([H, 2, 65], F32, tag="p_pa")
                    nc.vector.tensor_scalar_mul(out=pa[:, 0, :], in0=part_sb,
                                                scalar1=fAB[:, 0:1])
                    nc.vector.tensor_scalar_mul(out=pa[:, 1, :], in0=part_sb,
                                                scalar1=fAB[:, 1:2])
                    nc.sync.dma_start(out=pin[:], in_=pa)
                    if not SKIP_CC[0]:
                        nc.gpsimd.collective_compute(
                            "AllReduce", ALU.add,
                            replica_groups=[[0, 1, 2, 3, 4, 5, 6, 7]],
                            ins=[pin[:]], outs=[pout[:]])
            for g in range(3):
                w1_issue(g)

            _mark(nc, "E:attn")
            # ============= Phase E: windowed attention =====================
            with tc.tile_pool(name="ps_s", bufs=2, space="PSUM") as ps_s, \
                 tc.tile_pool(name="ps_pt", bufs=2, space="PSUM") as ps_pt, \
                 tc.tile_pool(name="ps_p0", bufs=1, space="PSUM") as ps_p0, \
                 tc.tile_pool(name="ps_o", bufs=1, space="PSUM") as ps_o, \
                 tc.tile_pool(name="ascr", bufs=2) as ascr:
                opr = [ps_o.tile([128, NQB, 128], F32, tag="o", name="o")]

                def attn_front(u):
                    pr, qb = u
                    s2 = ps_s.tile([128, 2, 512], F32, tag="s2")
                    for sub in range(2):
                        p0 = 64 * sub
                        qs = QT[p0:p0 + 64, pr, qb * 128:(qb + 1) * 128]
                        nc.tensor.matmul(
                            s2[:, sub, 0:WIN], qs,
                            KT[p0:p0 + 64, pr, qb * 128:qb * 128 + WIN],
                            start=True, stop=False)
                        nc.tensor.matmul(s2[:, sub, WIN:WIN + 1], qs,
                                         KT[p0:p0 + 64, pr, 640:641],
                                         start=False, stop=False)
                        nc.tensor.matmul(s2[:, sub, 0:WIN + 1], idb,
                                         msk[:, qb, :],
                                         start=False, stop=True)
                    return s2

                def attn_back(u, s2):
                    pr, qb = u
                    p1 = ascr.tile([128, 2, WIN + 1], BF16, tag="a_p1")
                    nc.scalar.activation(out=p1, in_=s2[:, :, 0:WIN + 1],
                                         func=AF.Exp, bias=neg3, scale=1.0)
                    rs = ascr.tile([128, 2], F32, tag="a_rs")
                    nc.vector.reduce_sum(out=rs, in_=p1, axis=AX.X)
                    rc = ascr.tile([128, 2], F32, tag="a_rc")
                    nc.vector.reciprocal(out=rc, in_=rs)
                    p8 = ascr.tile([128, 2, WIN + 1], BF16, tag="a_p8")
                    for sub in range(2):
                        nc.gpsimd.tensor_scalar_mul(
                            out=p8[:, sub], in0=p1[:, sub],
                            scalar1=rc[:, sub:sub + 1])
                    pt = ps_pt.tile([128, 2, 2, 128], BF16, tag="pt")
                    pt0 = ps_p0.tile([1, 2, 128], BF16, tag="pt0")
                    for sub in range(2):
                        nc.tensor.transpose(pt[:, sub, 0, :],
                                            p8[:, sub, 0:128], idb)
                        nc.tensor.transpose(pt[:, sub, 1, :],
                                            p8[:, sub, 128:256], idb)
                        nc.tensor.transpose(pt0[:, sub, :],
                                            p8[:, sub, WIN:WIN + 1], idb)
                    ptb = ascr.tile([128, 2, 2, 128], BF16, tag="a_ptb")
                    if qb % 2 == 0:
                        nc.vector.tensor_copy(out=ptb, in_=pt)
                    else:
                        nc.scalar.copy(out=ptb, in_=pt)
                    pt0b = ascr.tile([1, 2, 128], BF16, tag="a_pt0b")
                    if qb % 2 == 0:
                        nc.scalar.copy(out=pt0b, in_=pt0)
                    else:
                        nc.vector.tensor_copy(out=pt0b, in_=pt0)
                    o_ps = opr[0]
                    for sub in range(2):
                        p0 = 64 * sub
                        dv = slice(64 * (2 * pr + sub),
                                   64 * (2 * pr + sub) + 64)
                        nc.tensor.matmul(o_ps[p0:p0 + 64, qb, :],
                                         V[:, qb, dv], ptb[:, sub, 0, :],
                                         start=True, stop=False)
                        nc.tensor.matmul(o_ps[p0:p0 + 64, qb, :],
                                         V[:, qb + 1, dv], ptb[:, sub, 1, :],
                                         start=False, stop=False)
                        nc.tensor.matmul(o_ps[p0:p0 + 64, qb, :],
                                         V[0:1, 5, dv],
                                         pt0b[:, sub, :], start=False,
                                         stop=True)
                    if qb == NQB - 1:
                        if pr % 2 == 0:
                            nc.vector.tensor_copy(out=OT[:, pr, :],
                                                  in_=o_ps.rearrange(
                                                      "p a b -> p (a b)"))
                        else:
                            nc.scalar.copy(out=OT[:, pr, :],
                                           in_=o_ps.rearrange(
                                               "p a b -> p (a b)"))
                        opr[0] = ps_o.tile([128, NQB, 128], F32, tag="o", name="o")

                units = [(pr, qb) for pr in range(8) for qb in range(NQB)]
                pend = deque()
                for u in units:
                    pend.append((u, attn_front(u)))
                    if len(pend) >= 2:
                        attn_back(*pend.popleft())
                while pend:
                    attn_back(*pend.popleft())

            _mark(nc, "F:patch")
            # ============= Phase F: gather + patch global row ==============
            with tc.tile_pool(name="ps_tf", bufs=2, space="PSUM") as ps_tf:
                gath = small.tile([H, 2, 65], F32, tag="p_gath")
                nc.sync.dma_start(out=gath,
                                  in_=(pin if SKIP_CC[0] else pout)[:])
                vA = small.tile([H, 65], F32, tag="p_vA")
                nc.vector.tensor_scalar_mul(out=vA, in0=gath[:, 0, :],
                                            scalar1=fAB[:, 0:1])
                vB = small.tile([H, 65], F32, tag="p_vB")
                nc.vector.tensor_scalar_mul(out=vB, in0=gath[:, 1, :],
                                            scalar1=fAB[:, 1:2])
                val = small.tile([H, 65], F32, tag="p_val")
                nc.vector.tensor_add(out=val, in0=vA, in1=vB)
                recS = small.tile([H, 1], F32, tag="p_recS")
                nc.vector.reciprocal(out=recS, in_=val[:, 64:65])
                a47 = small.tile([H, HD], F32, tag="p_a47")
                nc.vector.tensor_scalar_mul(out=a47, in0=val[:, 0:64],
                                            scalar1=recS)
                a47t_ps = ps_tf.tile([HD, H], F32, tag="tf")
                nc.tensor.transpose(a47t_ps, a47, idf[0:H, 0:H])
                a47T = small.tile([HD, H], F8, tag="p_a47T")
                nc.scalar.copy(out=a47T, in_=a47t_ps)
                fix_sb = small.tile([128, 8], F8, tag="p_fix")
                a47v = a47T.rearrange("p (t two) -> p t two", two=2)
                nc.sync.dma_start(out=fix_sb[0:64, :], in_=a47v[:, :, 0])
                nc.sync.dma_start(out=fix_sb[64:128, :], in_=a47v[:, :, 1])
                for t in range(8):
                    nc.vector.copy_predicated(out=OT[:, t, CHUNK - 1:CHUNK],
                                              mask=fixsel,
                                              data=fix_sb[:, t:t + 1])

            _mark(nc, "G:wo")
            # ============= Phase G: out-proj (fp8 DR) + residual ===========
            with tc.tile_pool(name="ps_g", bufs=4, space="PSUM") as ps_g, \
                 tc.tile_pool(name="gscr", bufs=2) as gscr:
                for m in range(8):
                    msl = slice(m * 128, (m + 1) * 128)
                    pr_ps = ps_g.tile([128, CHUNK], F32, tag="g")
                    for t in range(0, 8, 2):
                        nc.tensor.matmul(pr_ps, wo8[:, t:t + 2, msl],
                                         OT[:, t:t + 2, :],
                                         start=t == 0, stop=t == 6,
                                         perf_mode=DR)
                    y1 = gscr.tile([128, CHUNK], F32, tag="evac512")
                    nc.scalar.activation(out=y1, in_=pr_ps, func=AF.Identity,
                                         bias=g_(4, m), scale=1.0 / WS)
                    nc.vector.tensor_add(out=yT[:, m, :], in0=y1,
                                         in1=xT[:, m, HALO:HALO + CHUNK])

        # ---- bigA closed: attention-era SBUF freed
        with tc.tile_pool(name="bigF", bufs=1) as bigF:
            ht = bigF.tile([128, 32, CHUNK], F8, tag="ht")
            w2lo = bigF.tile([128, 32, D], F8, tag="w2lo")

            _mark(nc, "H:ln2")
            # ============= Phase H: LN2 -> h2hi/h2lo (fp8) =================
            with tc.tile_pool(name="ps_row2", bufs=2, space="PSUM") as ps_row2, \
                 tc.tile_pool(name="ps_bc2", bufs=2, space="PSUM") as ps_bc2, \
                 tc.tile_pool(name="lnrow2", bufs=1) as lnrow2, \
                 tc.tile_pool(name="lnscr2", bufs=2) as lnscr2:
                def ln2_out(kt, sl, t2):
                    h2b = lnscr2.tile([128, CHUNK], BF16, tag="ln2_h2b")
                    nc.gpsimd.tensor_scalar(
                        out=h2b, in0=t2,
                        scalar1=g_(2, kt), scalar2=g_(3, kt),
                        op0=ALU.mult, op1=ALU.add)
                    nc.scalar.copy(out=h2hi[:, kt, :], in_=h2b)
                    nc.vector.tensor_sub(out=h2lo[:, kt, :], in0=h2b,
                                         in1=h2hi[:, kt, :])
                layernorm_T(yT, [(0, CHUNK)], ln2_out,
                            (ps_row2, ps_bc2, lnscr2, lnrow2))

            _mark(nc, "I:ffn1")
            # ============= Phase I: FFN1 (compensated fp8) -> ht ===========
            with tc.tile_pool(name="ps_f1", bufs=4, space="PSUM") as ps_f1:
                for g in range(8):
                    w1_issue(g + 3)
                    if g == 5:
                        for wg in range(4):
                            wgs = slice(wg * 8, (wg + 1) * 8)
                            nc.sync.dma_start(out=w2lo[:, wgs, :],
                                              in_=inp["w2lo"][:, wgs, :])
                    whi, wlo = w1sl[g]
                    for ml in range(4):
                        m = 4 * g + ml
                        msl = slice(ml * 128, (ml + 1) * 128)
                        h_ps = ps_f1.tile([128, CHUNK], F32, tag="f1")
                        for t in range(0, 8, 2):
                            nc.tensor.matmul(h_ps, whi[:, t:t + 2, msl],
                                             h2hi[:, t:t + 2, :],
                                             start=t == 0, stop=False,
                                             perf_mode=DR)
                            nc.tensor.matmul(h_ps, whi[:, t:t + 2, msl],
                                             h2lo[:, t:t + 2, :],
                                             start=False, stop=False,
                                             perf_mode=DR)
                            nc.tensor.matmul(h_ps, wlo[:, t:t + 2, msl],
                                             h2hi[:, t:t + 2, :],
                                             start=False, stop=t == 6,
                                             perf_mode=DR)
                        nc.scalar.activation(out=ht[:, m, :], in_=h_ps,
                                             func=AF.Gelu,
                                             bias=b1h[:, m:m + 1],
                                             scale=1.0 / WS)

            _mark(nc, "J:ffn2")
            # ============= Phase J: FFN2 (W-comp fp8) + residual + out =====
            with tc.tile_pool(name="ps_f2", bufs=4, space="PSUM") as ps_f2, \
                 tc.tile_pool(name="jscr", bufs=2) as jscr:
                for mo in range(8):
                    msl = slice(mo * 128, (mo + 1) * 128)
                    f2_ps = ps_f2.tile([128, CHUNK], F32, tag="f2")
                    for t in range(0, 32, 2):
                        nc.tensor.matmul(f2_ps, w2hi[:, t:t + 2, msl],
                                         ht[:, t:t + 2, :],
                                         start=t == 0, stop=False,
                                         perf_mode=DR)
                        nc.tensor.matmul(f2_ps, w2lo[:, t:t + 2, msl],
                                         ht[:, t:t + 2, :],
                                         start=False, stop=t == 30,
                                         perf_mode=DR)
                    f1 = jscr.tile([128, CHUNK], F32, tag="evac512")
                    nc.scalar.activation(out=f1, in_=f2_ps,
                                         func=AF.Identity,
                                         bias=g_(5, mo), scale=1.0 / WS)
                    om = jscr.tile([128, CHUNK], F32, tag="out_m")
                    nc.vector.tensor_add(out=om, in0=f1, in1=yT[:, mo, :])
                    nc.sync.dma_start(out=out_d[:, mo, :], in_=om)

# ------------------------------------------------------------------ driver --

_CACHE = {}


def _prep_core_inputs(inputs, c, shared_cache={}):
    bf = ml_dtypes.bfloat16
    f8 = ml_dtypes.float8_e4m3

    def q8(a):
        return np.ascontiguousarray(a).astype(f8)

    key = id(inputs.get("Wq"))
    shared = shared_cache.get(key)
    if shared is None:
        shared_cache.clear()
        w1t = _tileP(np.asarray(inputs["W1"], np.float32)) * WS  # [128,8,FF]
        w1hi = q8(w1t)
        w1lo = q8(w1t - w1hi.astype(np.float32))
        w2t = np.ascontiguousarray(
            np.asarray(inputs["W2"], np.float32)
            .reshape(32, 128, D).transpose(1, 0, 2)) * WS        # [128,32,D]
        w2hi = q8(w2t)
        w2lo = q8(w2t - w2hi.astype(np.float32))
        pv = np.stack([_vec_t(inputs[k]) for k in
                       ("ln1_g", "ln1_b", "ln2_g", "ln2_b", "bo", "b2")],
                      axis=1)                                     # [128,6,8]
        shared = {
            "wq8": q8(_tileP(np.asarray(inputs["Wq"], np.float32)) * WS),
            "wk8": q8(_tileP(np.asarray(inputs["Wk"], np.float32)) * WS),
            "wv8": q8(_tileP(np.asarray(inputs["Wv"], np.float32)) * WS),
            "wo8": q8(_tileP(np.asarray(inputs["Wo"], np.float32)) * WS),
            "w1hi": w1hi, "w1lo": w1lo, "w2hi": w2hi, "w2lo": w2lo,
            "pvec": np.ascontiguousarray(pv, dtype=np.float32),
            "b1h": np.ascontiguousarray(
                np.asarray(inputs["b1"], np.float32).reshape(32, 128).T),
        }
        shared_cache[key] = shared
    x = np.asarray(inputs["x"], np.float32)
    xT = np.ascontiguousarray(
        _make_x_ext(x, c).T.reshape(8, 128, NSLOT).transpose(1, 0, 2))
    msk = np.ascontiguousarray(
        _make_mask(c).transpose(1, 0, 2)).astype(bf)
    fs = np.full((128, 1), 1 if c % 4 == 3 else 0, np.uint8)
    fAB = np.zeros((16, 2), np.float32)
    fAB[:, 0] = 1.0 if c < 4 else 0.0
    fAB[:, 1] = 0.0 if c < 4 else 1.0
    return {**shared, "xT": xT, "msk": msk, "fixsel": fs, "fAB": fAB}


def get_nc():
    if "nc" not in _CACHE:
        _CACHE["nc"] = _build_nc()
    return _CACHE["nc"]


def kernel(**inputs):
    nc = get_nc()
    in_maps = [_prep_core_inputs(inputs, c) for c in range(N_CORES)]
    res = run_bass_kernel_spmd(nc, in_maps, core_ids=list(range(N_CORES)),
                               trace=False)
    out = np.zeros((B, T, D), np.float32)
    for c in range(N_CORES):
        b, j = divmod(c, 4)
        oT = res.results[c]["outT"]          # [128, 8, 512]
        out[b, j * CHUNK:(j + 1) * CHUNK] = \
            oT.transpose(1, 0, 2).reshape(D, CHUNK).T
    return out


# revision 24
# speedup vs baseline: 1.0280x; 1.0280x over previous
"""Longformer block on 8 TRN2 NeuronCores (Bass/Tile, SPMD).

Sharding: data-parallel over (batch, sequence): core c -> batch c//4, token
chunk (c%4)*512..+512. Weights replicated (fp8 e4m3, scaled x64, with an
fp8 residual-compensation tensor for W1/W2). On-chip layout is transposed
[D, token] so LN/residual/matmuls need no device transposes (host
pre-transposes x; LN stats via ones-vector f32r matmuls on PE).

Big GEMMs run as fp8 DoubleRow matmuls (two 128-deep k-tiles per
instruction). The FFN uses residual-compensated fp8:
  FFN1: W1hi@h_hi + W1lo@h_hi + W1hi@h_lo     (3 DoubleRows per 2 k-tiles)
  FFN2: W2hi@g + W2lo@g                        (2 DoubleRows per 2 k-tiles)
which keeps the fp8 quantization error at second order.

Attention: banded causal window (halo of 128 tokens recomputed locally) +
the token-0 global column as a 257th score column. The one global *row*
(token T-1) is computed via per-core exp-sum partials over each core's own
K/V slice, combined with an in-kernel AllReduce, and patched into the owning
core's output column with copy_predicated.
"""

from collections import deque

import numpy as np
import ml_dtypes

import concourse.bass as bass
import concourse.mybir as mybir
import concourse.tile as tile
from concourse.masks import make_identity
from concourse.bass_utils import run_bass_kernel_spmd

F32 = mybir.dt.float32
F32R = mybir.dt.float32r
BF16 = mybir.dt.bfloat16
F8 = mybir.dt.float8e4
AF = mybir.ActivationFunctionType
ALU = mybir.AluOpType
AX = mybir.AxisListType
DR = mybir.MatmulPerfMode.DoubleRow

D = 1024
FF = 4096
H = 16
HD = 64
T = 2048
B = 2
CHUNK = 512
HALO = 128
NSLOT = 768          # [halo 128 | own 512 | t0 | t2047 | pad]
NKV = 641            # slots 0..640 hold K/V (640 = token0); 641 = q2047 src
NQB = 4
WIN = 256
NEG = -1e30
EPS = 1e-5
N_CORES = 8
WS = 64.0            # fp8 weight pre-scale
SKIP_CC = [False]   # set kernel.SKIP_CC[0]=True to build without the
                    # collective (TimelineSim is single-core only)
PHASE_MARKS = []    # (phase_name, first_inst_id) filled during _emit


def _mark(nc, name):
    PHASE_MARKS.append((name, set(nc.inst_map.keys())))

# ---------------------------------------------------------------- bir fix ---

_waitfix_ctr = [0]


def _split_multiwaits(nc):
    """This container's walrus accepts ONE sync-wait per instruction; Tile
    attaches several. Hoist extras onto NoOps just before each instruction
    (Tile sems are monotonic within a context, so sequential waits are
    equivalent)."""
    n = 0
    for func in nc.m.functions:
        for bb in func.blocks:
            out = []
            changed = False
            for inst in bb.instructions:
                si = inst.sync_info
                if si is not None and len(si.on_wait) > 1:
                    waits = list(si.on_wait)
                    keep = [w for w in waits
                            if getattr(w, "wait_mode", "") not in
                            ("sem-ge-imm", "sem-ge-reg")]
                    if keep:
                        hoist = [w for w in waits if w not in keep]
                        last = keep
                    else:
                        hoist, last = waits[:-1], [waits[-1]]
                    for w in hoist:
                        _waitfix_ctr[0] += 1
                        nop = mybir.InstNoOp(name=f"I-waitfix-{_waitfix_ctr[0]}")
                        nop.engine = inst.engine
                        nop.sync_info = mybir.SyncInfo(on_wait=[w], on_update=[])
                        out.append(nop)
                        n += 1
                    si.on_wait = last
                    changed = True
                out.append(inst)
            if changed:
                bb.instructions[:] = out
    return n

# ------------------------------------------------------------ host helpers --


def _make_x_ext(x, c):
    b, j = divmod(c, 4)
    start = j * CHUNK
    ext = np.zeros((NSLOT, D), np.float32)
    ext[0:HALO] = x[b, start - HALO:start] if j > 0 else x[b, 0:HALO]
    ext[HALO:HALO + CHUNK] = x[b, start:start + CHUNK]
    ext[640] = x[b, 0]
    ext[641] = x[b, T - 1]
    return ext


def _make_mask(c):
    b, j = divmod(c, 4)
    start = j * CHUNK
    m = np.full((NQB, 128, WIN + 1), NEG, np.float32)
    il = np.arange(128)[:, None]
    jl = np.arange(WIN)[None, :]
    for qb in range(NQB):
        q_abs = start + qb * 128 + il
        slot = qb * 128 + jl
        band = (jl >= il) & (jl <= il + 128)
        valid = (j > 0) | (slot >= HALO)
        blk = m[qb, :, :WIN]
        blk[band & valid] = 0.0
        tok0_in_band = (q_abs[:, 0] <= HALO) & (j == 0)
        m[qb, :, WIN] = np.where(tok0_in_band, NEG, 0.0)
    return m


def _tileP(a, p=128):
    """[N*p, ...] -> [p, N, ...] partition-tiled layout."""
    n = a.shape[0] // p
    return np.ascontiguousarray(
        a.reshape(n, p, *a.shape[1:]).transpose(1, 0, *range(2, a.ndim + 1)))


def _vec_t(v):
    return np.ascontiguousarray(np.asarray(v, np.float32).reshape(-1, 128).T)

# ------------------------------------------------------------ bass program --


def _build_nc():
    nc = bass.Bass()

    f8 = mybir.dt
    inp = {}
    for name, shape, dt in [
        ("xT", [128, 8, NSLOT], F32),
        ("wq8", [128, 8, D], F8), ("wk8", [128, 8, D], F8),
        ("wv8", [128, 8, D], F8), ("wo8", [128, 8, D], F8),
        ("w1hi", [128, 8, FF], F8), ("w1lo", [128, 8, FF], F8),
        ("w2hi", [128, 32, D], F8), ("w2lo", [128, 32, D], F8),
        ("msk", [128, NQB, WIN + 1], BF16),
        ("pvec", [128, 6, 8], F32),       # g1,b1,g2,b2,bo,b2
        ("b1h", [128, 32], F32),
        ("fixsel", [128, 1], mybir.dt.uint8),
        ("fAB", [16, 2], F32),
    ]:
        inp[name] = nc.dram_tensor(name, shape, dt, kind="ExternalInput")
    out_d = nc.dram_tensor("outT", [128, 8, CHUNK], F32, kind="ExternalOutput")
    pin = nc.dram_tensor("pin", [H, 2, HD + 1], F32)
    pout = nc.dram_tensor("pout", [H, 2, HD + 1], F32, addr_space="Shared")

    with tile.TileContext(nc) as tc:
        _emit(nc, tc, inp, out_d, pin, pout)
    _split_multiwaits(nc)
    return nc


def _emit(nc, tc, inp, out_d, pin, pout):
    from contextlib import ExitStack
    ctx = ExitStack()
    with ctx:
        pers = ctx.enter_context(tc.tile_pool(name="pers", bufs=1))
        small = ctx.enter_context(tc.tile_pool(name="small", bufs=2))
        bigY = ctx.enter_context(tc.tile_pool(name="bigY", bufs=1))
        big0 = ctx.enter_context(tc.tile_pool(name="big0", bufs=1))
        w1p = ctx.enter_context(tc.tile_pool(name="w1p", bufs=3))

        # ---- persistent constants / params
        pvec = pers.tile([128, 6, 8], F32, tag="pvec", name="pvec")
        b1h = pers.tile([128, 32], F32, tag="b1h", name="b1h")
        msk = pers.tile([128, NQB, WIN + 1], BF16, tag="msk")
        fixsel = pers.tile([128, 1], mybir.dt.uint8, tag="fixsel")
        fAB = pers.tile([16, 2], F32, tag="fAB")

        idf = pers.tile([128, 128], F32, tag="idf")
        idb = pers.tile([128, 128], BF16, tag="idb")
        onesp = pers.tile([128, 1], BF16, tag="onesp")
        epst = pers.tile([1, 1], F32, tag="epst")
        neg3 = pers.tile([128, 1], F32, tag="neg3")

        yT = bigY.tile([128, 8, CHUNK], F32, tag="yT")
        h2hi = bigY.tile([128, 8, CHUNK], F8, tag="h2hi")
        h2lo = bigY.tile([128, 8, CHUNK], F8, tag="h2lo")
        w2hi = big0.tile([128, 32, D], F8, tag="w2hi")
        w1sl = {}   # FFN1 weight slab stream: g -> (hi, lo), 4 m-tiles each

        def w1_issue(g):
            if g >= 8:
                return
            hi = w1p.tile([128, 8, 512], F8, tag="w1hi")
            lo = w1p.tile([128, 8, 512], F8, tag="w1lo")
            gs = slice(g * 512, (g + 1) * 512)
            nc.sync.dma_start(out=hi, in_=inp["w1hi"][:, :, gs])
            nc.sync.dma_start(out=lo, in_=inp["w1lo"][:, :, gs])
            w1sl[g] = (hi, lo)

        def g_(i, kt):   # per-partition scalar views of packed params
            return pvec[:, i, kt:kt + 1]

        # ================= LN in transposed layout (bf16 stats) ============
        def layernorm_T(src, chunks, emit_out, pools):
            """emit_out(kt, sl, t2): consume normalized bf16 (pre-g/b).
            chunks: list of (start, end) column ranges to process."""
            ps_row, ps_bc, scr, rowp = pools
            width = max(e for _, e in chunks)
            mus = []
            for (cs, ce) in chunks:
                mus.append((ps_row.tile([1, ce - cs], F32, tag="row",
                                        name="mu"),
                            ps_row.tile([1, ce - cs], F32, tag="row",
                                        name="msq")))
            onesb = scr.tile([128, 1], BF16, tag="ln_onesb")
            nc.vector.memset(onesb, 1.0 / D)
            ones1b = scr.tile([1, 128], BF16, tag="ln_ones1b")
            nc.vector.memset(ones1b, 1.0)
            for kt in range(8):
                xb = scr.tile([128, width], BF16, tag="ln_xb")
                nc.scalar.copy(out=xb, in_=src[:, kt, 0:width])
                xsq = scr.tile([128, width], BF16, tag="ln_xsq")
                nc.vector.tensor_mul(out=xsq, in0=xb, in1=xb)
                for cch, (cs, ce) in enumerate(chunks):
                    sl = slice(cs, ce)
                    nc.tensor.matmul(mus[cch][0], onesb, xb[:, sl],
                                     start=kt == 0, stop=kt == 7)
                    nc.tensor.matmul(mus[cch][1], onesb, xsq[:, sl],
                                     start=kt == 0, stop=kt == 7)
            bcs = []
            for cch, (cs, ce) in enumerate(chunks):
                cw = ce - cs
                mu_ps, msq_ps = mus[cch]
                musb = rowp.tile([1, cw], F32, tag="ln_mu")
                nc.scalar.copy(out=musb, in_=mu_ps)
                tmp = rowp.tile([1, cw], F32, tag="ln_tmp")
                nc.vector.tensor_mul(out=tmp, in0=musb, in1=musb)
                nc.vector.tensor_sub(out=tmp, in0=msq_ps, in1=tmp)
                nc.scalar.activation(out=tmp, in_=tmp, func=AF.Sqrt,
                                     bias=epst, scale=1.0)
                nc.vector.reciprocal(out=tmp, in_=tmp)       # rstd
                tmpb = rowp.tile([1, cw], BF16, tag="ln_tmpb")
                nc.vector.tensor_copy(out=tmpb, in_=tmp)
                nc.vector.tensor_mul(out=musb, in0=musb, in1=tmp)
                musbb = rowp.tile([1, cw], BF16, tag="ln_musbb")
                nc.scalar.mul(out=musbb, in_=musb, mul=-1.0)  # -mu*rstd
                rb_ps = ps_bc.tile([128, cw], F32, tag="bc", name="rb")
                nc.tensor.matmul(rb_ps, ones1b, tmpb, start=True, stop=True)
                nb_ps = ps_bc.tile([128, cw], F32, tag="bc", name="nb")
                nc.tensor.matmul(nb_ps, ones1b, musbb, start=True, stop=True)
                rb_sb = scr.tile([128, cw], BF16, tag="ln_rb")
                nc.scalar.copy(out=rb_sb, in_=rb_ps)
                nb_sb = scr.tile([128, cw], BF16, tag="ln_nb")
                nc.scalar.copy(out=nb_sb, in_=nb_ps)
                bcs.append((rb_sb, nb_sb))
            for kt in range(8):
                for cch, (cs, ce) in enumerate(chunks):
                    sl = slice(cs, ce)
                    rb_sb, nb_sb = bcs[cch]
                    t1 = scr.tile([128, ce - cs], BF16, tag="ln_t1")
                    nc.vector.tensor_mul(out=t1, in0=src[:, kt, sl],
                                         in1=rb_sb)
                    t2 = scr.tile([128, ce - cs], BF16, tag="ln_t2")
                    nc.vector.tensor_add(out=t2, in0=t1, in1=nb_sb)
                    emit_out(kt, sl, t2)

        with tc.tile_pool(name="bigA", bufs=1) as bigA:
            # ---- long-lived activations (until end of phase G)
            xT = bigA.tile([128, 8, NSLOT], F32, tag="xT")
            nc.sync.dma_start(out=xT[:, 0, 0:384], in_=inp["xT"][:, 0, 0:384])
            nc.sync.dma_start(out=xT[:, 0, 384:NSLOT],
                              in_=inp["xT"][:, 0, 384:NSLOT])
            for kt in range(1, 8):
                nc.sync.dma_start(out=xT[:, kt, :], in_=inp["xT"][:, kt, :])
            nc.sync.dma_start(out=pvec, in_=inp["pvec"][:])
            nc.sync.dma_start(out=b1h, in_=inp["b1h"][:])
            nc.sync.dma_start(out=msk, in_=inp["msk"][:])
            nc.sync.dma_start(out=fixsel, in_=inp["fixsel"][:])
            nc.sync.dma_start(out=fAB, in_=inp["fAB"][:])
            make_identity(nc, idf)
            make_identity(nc, idb)
            nc.vector.memset(onesp, 1.0)
            nc.vector.memset(epst, EPS)
            nc.vector.memset(neg3, -3.0)

            wo8 = bigA.tile([128, 8, D], F8, tag="wo8")
            h8 = bigA.tile([128, 8, NSLOT], F8, tag="h8")
            QT = bigA.tile([128, 8, CHUNK], BF16, tag="QT")
            q47T = bigA.tile([128, 8], BF16, tag="q47T")
            KT = bigA.tile([128, 8, NKV], BF16, tag="KT")
            V = bigA.tile([128, 6, D], BF16, tag="V")
            OT = bigA.tile([128, 8, CHUNK], F8, tag="OT")

            with tc.tile_pool(name="bigW", bufs=1) as bigW:
                wq8 = bigW.tile([128, 8, D], F8, tag="wq8")
                nc.sync.dma_start(out=wq8[:, :, 0:512],
                                  in_=inp["wq8"][:, :, 0:512])
                wk8 = bigW.tile([128, 8, D], F8, tag="wk8")
                nc.sync.dma_start(out=wk8[:, :, 0:512],
                                  in_=inp["wk8"][:, :, 0:512])
                nc.sync.dma_start(out=wq8[:, :, 512:D],
                                  in_=inp["wq8"][:, :, 512:D])
                nc.sync.dma_start(out=wk8[:, :, 512:D],
                                  in_=inp["wk8"][:, :, 512:D])
                wv8 = bigW.tile([128, 8, D], F8, tag="wv8")
                nc.sync.dma_start(out=wv8, in_=inp["wv8"][:])
                nc.sync.dma_start(out=wo8, in_=inp["wo8"][:])
                for g in range(4):
                    gs = slice(g * 8, (g + 1) * 8)
                    nc.sync.dma_start(out=w2hi[:, gs, :],
                                      in_=inp["w2hi"][:, gs, :])

                _mark(nc, "B:ln1")
                # ============= Phase B: LN1 -> h8 (fp8) ====================
                with tc.tile_pool(name="ps_row1", bufs=4, space="PSUM") as ps_row, \
                     tc.tile_pool(name="ps_bc1", bufs=4, space="PSUM") as ps_bc, \
                     tc.tile_pool(name="lnrow1", bufs=1) as lnrow, \
                     tc.tile_pool(name="lnscr1", bufs=2) as lnscr:
                    def ln1_out(kt, sl, t2):
                        nc.gpsimd.tensor_scalar(
                            out=h8[:, kt, sl], in0=t2,
                            scalar1=g_(0, kt), scalar2=g_(1, kt),
                            op0=ALU.mult, op1=ALU.add)
                    layernorm_T(xT, [(0, 384), (384, 642)], ln1_out,
                                (ps_row, ps_bc, lnscr, lnrow))

                _mark(nc, "C:qkv")
                # ============= Phase C: QKV (fp8 DoubleRow) ================
                with tc.tile_pool(name="ps_big", bufs=6, space="PSUM") as ps_big, \
                     tc.tile_pool(name="ps_tiny", bufs=2, space="PSUM") as ps_tiny:
                    for mp in range(4):
                        ms = [2 * mp, 2 * mp + 1]
                        qp, kp, k2p, q47p = {}, {}, {}, {}
                        for m in ms:
                            qp[m] = ps_big.tile([128, CHUNK], F32,
                                                tag="big", name="q_ps")
                            kp[m] = ps_big.tile([128, 512], F32,
                                                tag="big", name="k_ps")
                            k2p[m] = ps_big.tile([128, NKV - 512], F32,
                                                 tag="big", name="k2_ps")
                            q47p[m] = ps_tiny.tile([128, 1], F32, tag="tiny",
                                                   name="q47_ps")
                        for t in range(0, 8, 2):
                            for m in ms:
                                msl = slice(m * 128, (m + 1) * 128)
                                nc.tensor.matmul(
                                    qp[m], wq8[:, t:t + 2, msl],
                                    h8[:, t:t + 2, HALO:HALO + CHUNK],
                                    start=t == 0, stop=t == 6, perf_mode=DR)
                                nc.tensor.matmul(
                                    kp[m], wk8[:, t:t + 2, msl],
                                    h8[:, t:t + 2, 0:512],
                                    start=t == 0, stop=t == 6, perf_mode=DR)
                                nc.tensor.matmul(
                                    k2p[m], wk8[:, t:t + 2, msl],
                                    h8[:, t:t + 2, 512:NKV],
                                    start=t == 0, stop=t == 6, perf_mode=DR)
                                nc.tensor.matmul(
                                    q47p[m], wq8[:, t, msl],
                                    h8[:, t, 641:642],
                                    start=t == 0, stop=False)
                                nc.tensor.matmul(
                                    q47p[m], wq8[:, t + 1, msl],
                                    h8[:, t + 1, 641:642],
                                    start=False, stop=t == 6)
                        for m in ms:
                            nc.vector.tensor_scalar_mul(
                                out=QT[:, m, :], in0=qp[m],
                                scalar1=1.0 / (WS * np.sqrt(HD)))
                            nc.scalar.mul(out=q47T[:, m:m + 1], in_=q47p[m],
                                          mul=1.0 / (WS * np.sqrt(HD)))
                            nc.scalar.mul(out=KT[:, m, 0:512], in_=kp[m],
                                          mul=1.0 / WS)
                            nc.vector.tensor_scalar_mul(
                                out=KT[:, m, 512:NKV], in0=k2p[m],
                                scalar1=1.0 / WS)
                    for tt in range(6):
                        for cch in range(2):
                            csl = slice(cch * 512, (cch + 1) * 512)
                            v_ps = ps_big.tile([128, 512], F32, tag="big")
                            for t in range(0, 8, 2):
                                nc.tensor.matmul(
                                    v_ps,
                                    h8[:, t:t + 2, tt * 128:(tt + 1) * 128],
                                    wv8[:, t:t + 2, csl],
                                    start=t == 0, stop=t == 6, perf_mode=DR)
                            nc.scalar.mul(out=V[:, tt, csl][:, 0:256],
                                          in_=v_ps[:, 0:256], mul=1.0 / WS)
                            nc.vector.tensor_scalar_mul(
                                out=V[:, tt, csl][:, 256:512],
                                in0=v_ps[:, 256:512], scalar1=1.0 / WS)

                    _mark(nc, "D:partials")
                    # ========= Phase D: global-row partials + AllReduce ====
                    sT = small.tile([128, H * 4], F32, tag="p_sT")
                    for h in range(H):
                        p0 = 64 * (h % 2)
                        s47_ps = ps_tiny.tile([128, 4], F32, tag="tiny")
                        for i in range(4):
                            nc.tensor.matmul(
                                s47_ps[:, i:i + 1],
                                KT[p0:p0 + 64, h // 2,
                                   HALO + 128 * i:HALO + 128 * (i + 1)],
                                q47T[p0:p0 + 64, h // 2:h // 2 + 1],
                                start=True, stop=True)
                        nc.scalar.copy(out=sT[:, 4 * h:4 * h + 4], in_=s47_ps)
                    p47 = small.tile([128, H * 4], BF16, tag="p_p47")
                    nc.scalar.activation(out=p47, in_=sT, func=AF.Exp)
                    ssum_ps = ps_tiny.tile([1, H * 4], F32, tag="tiny")
                    nc.tensor.matmul(ssum_ps, onesp, p47, start=True,
                                     stop=True)
                    s_c = small.tile([1, H], F32, tag="p_sc")
                    nc.vector.reduce_sum(
                        out=s_c,
                        in_=ssum_ps.rearrange("p (h i) -> p h i", i=4),
                        axis=AX.X)
                    oall = small.tile([65, H], F32, tag="p_oall")
                    for h in range(H):
                        o47_ps = ps_tiny.tile([64, 1], F32, tag="tiny")
                        for i in range(4):
                            nc.tensor.matmul(
                                o47_ps, V[:, 1 + i, 64 * h:64 * h + 64],
                                p47[:, 4 * h + i:4 * h + i + 1],
                                start=i == 0, stop=i == 3)
                        nc.scalar.copy(out=oall[0:64, h:h + 1], in_=o47_ps)
                    nc.sync.dma_start(out=oall[64:65, :], in_=s_c)
                    part_ps = ps_tiny.tile([H, 65], F32, tag="tiny")
                    nc.tensor.transpose(part_ps, oall, idf[0:65, 0:65])
                    part_sb = small.tile([H, 65], F32, tag="p_part")
                    nc.scalar.copy(out=part_sb, in_=part_ps)
                    pa = small.tile([H, 2, 65], F32, tag="p_pa")
                    nc.vector.tensor_scalar_mul(out=pa[:, 0, :], in0=part_sb,
                                                scalar1=fAB[:, 0:1])
                    nc.vector.tensor_scalar_mul(out=pa[:, 1, :], in0=part_sb,
                                                scalar1=fAB[:, 1:2])
                    nc.sync.dma_start(out=pin[:], in_=pa)
                    if not SKIP_CC[0]:
                        nc.gpsimd.collective_compute(
                            "AllReduce", ALU.add,
                            replica_groups=[[0, 1, 2, 3, 4, 5, 6, 7]],
                            ins=[pin[:]], outs=[pout[:]])
            for g in range(3):
                w1_issue(g)

            _mark(nc, "E:attn")
            # ============= Phase E: windowed attention =====================
            with tc.tile_pool(name="ps_s", bufs=2, space="PSUM") as ps_s, \
                 tc.tile_pool(name="ps_pt", bufs=2, space="PSUM") as ps_pt, \
                 tc.tile_pool(name="ps_p0", bufs=1, space="PSUM") as ps_p0, \
                 tc.tile_pool(name="ps_o", bufs=1, space="PSUM") as ps_o, \
                 tc.tile_pool(name="ascr", bufs=3) as ascr:
                opr = [ps_o.tile([128, NQB, 128], F32, tag="o", name="o")]

                def attn_front(u):
                    pr, qb = u
                    s2 = ps_s.tile([128, 2, 512], F32, tag="s2")
                    for sub in range(2):
                        p0 = 64 * sub
                        qs = QT[p0:p0 + 64, pr, qb * 128:(qb + 1) * 128]
                        nc.tensor.matmul(
                            s2[:, sub, 0:WIN], qs,
                            KT[p0:p0 + 64, pr, qb * 128:qb * 128 + WIN],
                            start=True, stop=False)
                        nc.tensor.matmul(s2[:, sub, WIN:WIN + 1], qs,
                                         KT[p0:p0 + 64, pr, 640:641],
                                         start=False, stop=False)
                        nc.tensor.matmul(s2[:, sub, 0:WIN + 1], idb,
                                         msk[:, qb, :],
                                         start=False, stop=True)
                    return s2

                def attn_back(u, s2):
                    pr, qb = u
                    p1 = ascr.tile([128, 2, WIN + 1], BF16, tag="a_p1")
                    nc.scalar.activation(out=p1, in_=s2[:, :, 0:WIN + 1],
                                         func=AF.Exp, bias=neg3, scale=1.0)
                    rs = ascr.tile([128, 2], F32, tag="a_rs")
                    nc.vector.reduce_sum(out=rs, in_=p1, axis=AX.X)
                    rc = ascr.tile([128, 2], F32, tag="a_rc")
                    nc.vector.reciprocal(out=rc, in_=rs)
                    p8 = ascr.tile([128, 2, WIN + 1], BF16, tag="a_p8")
                    for sub in range(2):
                        nc.gpsimd.tensor_scalar_mul(
                            out=p8[:, sub], in0=p1[:, sub],
                            scalar1=rc[:, sub:sub + 1])
                    pt = ps_pt.tile([128, 2, 2, 128], BF16, tag="pt")
                    pt0 = ps_p0.tile([1, 2, 128], BF16, tag="pt0")
                    for sub in range(2):
                        nc.tensor.transpose(pt[:, sub, 0, :],
                                            p8[:, sub, 0:128], idb)
                        nc.tensor.transpose(pt[:, sub, 1, :],
                                            p8[:, sub, 128:256], idb)
                        nc.tensor.transpose(pt0[:, sub, :],
                                            p8[:, sub, WIN:WIN + 1], idb)
                    ptb = ascr.tile([128, 2, 2, 128], BF16, tag="a_ptb")
                    if qb % 2 == 0:
                        nc.vector.tensor_copy(out=ptb, in_=pt)
                    else:
                        nc.scalar.copy(out=ptb, in_=pt)
                    pt0b = ascr.tile([1, 2, 128], BF16, tag="a_pt0b")
                    if qb % 2 == 0:
                        nc.scalar.copy(out=pt0b, in_=pt0)
                    else:
                        nc.vector.tensor_copy(out=pt0b, in_=pt0)
                    o_ps = opr[0]
                    for sub in range(2):
                        p0 = 64 * sub
                        dv = slice(64 * (2 * pr + sub),
                                   64 * (2 * pr + sub) + 64)
                        nc.tensor.matmul(o_ps[p0:p0 + 64, qb, :],
                                         V[:, qb, dv], ptb[:, sub, 0, :],
                                         start=True, stop=False)
                        nc.tensor.matmul(o_ps[p0:p0 + 64, qb, :],
                                         V[:, qb + 1, dv], ptb[:, sub, 1, :],
                                         start=False, stop=False)
                        nc.tensor.matmul(o_ps[p0:p0 + 64, qb, :],
                                         V[0:1, 5, dv],
                                         pt0b[:, sub, :], start=False,
                                         stop=True)
                    if qb == NQB - 1:
                        if pr % 2 == 0:
                            nc.vector.tensor_copy(out=OT[:, pr, :],
                                                  in_=o_ps.rearrange(
                                                      "p a b -> p (a b)"))
                        else:
                            nc.scalar.copy(out=OT[:, pr, :],
                                           in_=o_ps.rearrange(
                                               "p a b -> p (a b)"))
                        opr[0] = ps_o.tile([128, NQB, 128], F32, tag="o", name="o")

                units = [(pr, qb) for pr in range(8) for qb in range(NQB)]
                pend = deque()
                for u in units:
                    pend.append((u, attn_front(u)))
                    if len(pend) >= 2:
                        attn_back(*pend.popleft())
                while pend:
                    attn_back(*pend.popleft())

            _mark(nc, "F:patch")
            # ============= Phase F: gather + patch global row ==============
            with tc.tile_pool(name="ps_tf", bufs=2, space="PSUM") as ps_tf:
                gath = small.tile([H, 2, 65], F32, tag="p_gath")
                nc.sync.dma_start(out=gath,
                                  in_=(pin if SKIP_CC[0] else pout)[:])
                vA = small.tile([H, 65], F32, tag="p_vA")
                nc.vector.tensor_scalar_mul(out=vA, in0=gath[:, 0, :],
                                            scalar1=fAB[:, 0:1])
                vB = small.tile([H, 65], F32, tag="p_vB")
                nc.vector.tensor_scalar_mul(out=vB, in0=gath[:, 1, :],
                                            scalar1=fAB[:, 1:2])
                val = small.tile([H, 65], F32, tag="p_val")
                nc.vector.tensor_add(out=val, in0=vA, in1=vB)
                recS = small.tile([H, 1], F32, tag="p_recS")
                nc.vector.reciprocal(out=recS, in_=val[:, 64:65])
                a47 = small.tile([H, HD], F32, tag="p_a47")
                nc.vector.tensor_scalar_mul(out=a47, in0=val[:, 0:64],
                                            scalar1=recS)
                a47t_ps = ps_tf.tile([HD, H], F32, tag="tf")
                nc.tensor.transpose(a47t_ps, a47, idf[0:H, 0:H])
                a47T = small.tile([HD, H], F8, tag="p_a47T")
                nc.scalar.copy(out=a47T, in_=a47t_ps)
                fix_sb = small.tile([128, 8], F8, tag="p_fix")
                a47v = a47T.rearrange("p (t two) -> p t two", two=2)
                nc.sync.dma_start(out=fix_sb[0:64, :], in_=a47v[:, :, 0])
                nc.sync.dma_start(out=fix_sb[64:128, :], in_=a47v[:, :, 1])
                for t in range(8):
                    nc.vector.copy_predicated(out=OT[:, t, CHUNK - 1:CHUNK],
                                              mask=fixsel,
                                              data=fix_sb[:, t:t + 1])

            _mark(nc, "G:wo")
            # ============= Phase G: out-proj (fp8 DR) + residual ===========
            with tc.tile_pool(name="ps_g", bufs=4, space="PSUM") as ps_g, \
                 tc.tile_pool(name="gscr", bufs=2) as gscr:
                for m in range(8):
                    msl = slice(m * 128, (m + 1) * 128)
                    pr_ps = ps_g.tile([128, CHUNK], F32, tag="g")
                    for t in range(0, 8, 2):
                        nc.tensor.matmul(pr_ps, wo8[:, t:t + 2, msl],
                                         OT[:, t:t + 2, :],
                                         start=t == 0, stop=t == 6,
                                         perf_mode=DR)
                    y1 = gscr.tile([128, CHUNK], F32, tag="evac512")
                    nc.scalar.activation(out=y1, in_=pr_ps, func=AF.Identity,
                                         bias=g_(4, m), scale=1.0 / WS)
                    nc.vector.tensor_add(out=yT[:, m, :], in0=y1,
                                         in1=xT[:, m, HALO:HALO + CHUNK])

        # ---- bigA closed: attention-era SBUF freed
        with tc.tile_pool(name="bigF", bufs=1) as bigF:
            ht = bigF.tile([128, 32, CHUNK], F8, tag="ht")
            w2lo = bigF.tile([128, 32, D], F8, tag="w2lo")

            _mark(nc, "H:ln2")
            # ============= Phase H: LN2 -> h2hi/h2lo (fp8) =================
            with tc.tile_pool(name="ps_row2", bufs=2, space="PSUM") as ps_row2, \
                 tc.tile_pool(name="ps_bc2", bufs=2, space="PSUM") as ps_bc2, \
                 tc.tile_pool(name="lnrow2", bufs=1) as lnrow2, \
                 tc.tile_pool(name="lnscr2", bufs=2) as lnscr2:
                def ln2_out(kt, sl, t2):
                    h2b = lnscr2.tile([128, CHUNK], BF16, tag="ln2_h2b")
                    nc.gpsimd.tensor_scalar(
                        out=h2b, in0=t2,
                        scalar1=g_(2, kt), scalar2=g_(3, kt),
                        op0=ALU.mult, op1=ALU.add)
                    nc.scalar.copy(out=h2hi[:, kt, :], in_=h2b)
                    nc.vector.tensor_sub(out=h2lo[:, kt, :], in0=h2b,
                                         in1=h2hi[:, kt, :])
                layernorm_T(yT, [(0, CHUNK)], ln2_out,
                            (ps_row2, ps_bc2, lnscr2, lnrow2))

            _mark(nc, "I:ffn1")
            # ============= Phase I: FFN1 (compensated fp8) -> ht ===========
            with tc.tile_pool(name="ps_f1", bufs=4, space="PSUM") as ps_f1:
                for g in range(8):
                    w1_issue(g + 3)
                    if g == 5:
                        for wg in range(4):
                            wgs = slice(wg * 8, (wg + 1) * 8)
                            nc.sync.dma_start(out=w2lo[:, wgs, :],
                                              in_=inp["w2lo"][:, wgs, :])
                    whi, wlo = w1sl[g]
                    for ml in range(4):
                        m = 4 * g + ml
                        msl = slice(ml * 128, (ml + 1) * 128)
                        h_ps = ps_f1.tile([128, CHUNK], F32, tag="f1")
                        for t in range(0, 8, 2):
                            nc.tensor.matmul(h_ps, whi[:, t:t + 2, msl],
                                             h2hi[:, t:t + 2, :],
                                             start=t == 0, stop=False,
                                             perf_mode=DR)
                            nc.tensor.matmul(h_ps, whi[:, t:t + 2, msl],
                                             h2lo[:, t:t + 2, :],
                                             start=False, stop=False,
                                             perf_mode=DR)
                            nc.tensor.matmul(h_ps, wlo[:, t:t + 2, msl],
                                             h2hi[:, t:t + 2, :],
                                             start=False, stop=t == 6,
                                             perf_mode=DR)
                        nc.scalar.activation(out=ht[:, m, :], in_=h_ps,
                                             func=AF.Gelu,
                                             bias=b1h[:, m:m + 1],
                                             scale=1.0 / WS)

            _mark(nc, "J:ffn2")
            # ============= Phase J: FFN2 (W-comp fp8) + residual + out =====
            with tc.tile_pool(name="ps_f2", bufs=4, space="PSUM") as ps_f2, \
                 tc.tile_pool(name="jscr", bufs=2) as jscr:
                for mo in range(8):
                    msl = slice(mo * 128, (mo + 1) * 128)
                    f2_ps = ps_f2.tile([128, CHUNK], F32, tag="f2")
                    for t in range(0, 32, 2):
                        nc.tensor.matmul(f2_ps, w2hi[:, t:t + 2, msl],
                                         ht[:, t:t + 2, :],
                                         start=t == 0, stop=False,
                                         perf_mode=DR)
                        nc.tensor.matmul(f2_ps, w2lo[:, t:t + 2, msl],
                                         ht[:, t:t + 2, :],
                                         start=False, stop=t == 30,
                                         perf_mode=DR)
                    f1 = jscr.tile([128, CHUNK], F32, tag="evac512")
                    nc.scalar.activation(out=f1, in_=f2_ps,
                                         func=AF.Identity,
                                         bias=g_(5, mo), scale=1.0 / WS)
                    om = jscr.tile([128, CHUNK], F32, tag="out_m")
                    nc.vector.tensor_add(out=om, in0=f1, in1=yT[:, mo, :])
                    nc.sync.dma_start(out=out_d[:, mo, :], in_=om)

# ------------------------------------------------------------------ driver --

_CACHE = {}


def _prep_core_inputs(inputs, c, shared_cache={}):
    bf = ml_dtypes.bfloat16
    f8 = ml_dtypes.float8_e4m3

    def q8(a):
        return np.ascontiguousarray(a).astype(f8)

    key = id(inputs.get("Wq"))
    shared = shared_cache.get(key)
    if shared is None:
        shared_cache.clear()
        w1t = _tileP(np.asarray(inputs["W1"], np.float32)) * WS  # [128,8,FF]
        w1hi = q8(w1t)
        w1lo = q8(w1t - w1hi.astype(np.float32))
        w2t = np.ascontiguousarray(
            np.asarray(inputs["W2"], np.float32)
            .reshape(32, 128, D).transpose(1, 0, 2)) * WS        # [128,32,D]
        w2hi = q8(w2t)
        w2lo = q8(w2t - w2hi.astype(np.float32))
        pv = np.stack([_vec_t(inputs[k]) for k in
                       ("ln1_g", "ln1_b", "ln2_g", "ln2_b", "bo", "b2")],
                      axis=1)                                     # [128,6,8]
        shared = {
            "wq8": q8(_tileP(np.asarray(inputs["Wq"], np.float32)) * WS),
            "wk8": q8(_tileP(np.asarray(inputs["Wk"], np.float32)) * WS),
            "wv8": q8(_tileP(np.asarray(inputs["Wv"], np.float32)) * WS),
            "wo8": q8(_tileP(np.asarray(inputs["Wo"], np.float32)) * WS),
            "w1hi": w1hi, "w1lo": w1lo, "w2hi": w2hi, "w2lo": w2lo,
            "pvec": np.ascontiguousarray(pv, dtype=np.float32),
            "b1h": np.ascontiguousarray(
                np.asarray(inputs["b1"], np.float32).reshape(32, 128).T),
        }
        shared_cache[key] = shared
    x = np.asarray(inputs["x"], np.float32)
    xT = np.ascontiguousarray(
        _make_x_ext(x, c).T.reshape(8, 128, NSLOT).transpose(1, 0, 2))
    msk = np.ascontiguousarray(
        _make_mask(c).transpose(1, 0, 2)).astype(bf)
    fs = np.full((128, 1), 1 if c % 4 == 3 else 0, np.uint8)
    fAB = np.zeros((16, 2), np.float32)
    fAB[:, 0] = 1.0 if c < 4 else 0.0
    fAB[:, 1] = 0.0 if c < 4 else 1.0
    return {**shared, "xT": xT, "msk": msk, "fixsel": fs, "fAB": fAB}


def get_nc():
    if "nc" not in _CACHE:
        _CACHE["nc"] = _build_nc()
    return _CACHE["nc"]


def kernel(**inputs):
    nc = get_nc()
    in_maps = [_prep_core_inputs(inputs, c) for c in range(N_CORES)]
    res = run_bass_kernel_spmd(nc, in_maps, core_ids=list(range(N_CORES)),
                               trace=False)
    out = np.zeros((B, T, D), np.float32)
    for c in range(N_CORES):
        b, j = divmod(c, 4)
        oT = res.results[c]["outT"]          # [128, 8, 512]
        out[b, j * CHUNK:(j + 1) * CHUNK] = \
            oT.transpose(1, 0, 2).reshape(D, CHUNK).T
    return out


# revision 25
# speedup vs baseline: 1.0526x; 1.0240x over previous
"""Longformer block on 8 TRN2 NeuronCores (Bass/Tile, SPMD).

Sharding: data-parallel over (batch, sequence): core c -> batch c//4, token
chunk (c%4)*512..+512. Weights replicated (fp8 e4m3, scaled x64, with an
fp8 residual-compensation tensor for W1/W2). On-chip layout is transposed
[D, token] so LN/residual/matmuls need no device transposes (host
pre-transposes x; LN stats via ones-vector f32r matmuls on PE).

Big GEMMs run as fp8 DoubleRow matmuls (two 128-deep k-tiles per
instruction). The FFN uses residual-compensated fp8:
  FFN1: W1hi@h_hi + W1lo@h_hi + W1hi@h_lo     (3 DoubleRows per 2 k-tiles)
  FFN2: W2hi@g + W2lo@g                        (2 DoubleRows per 2 k-tiles)
which keeps the fp8 quantization error at second order.

Attention: banded causal window (halo of 128 tokens recomputed locally) +
the token-0 global column as a 257th score column. The one global *row*
(token T-1) is computed via per-core exp-sum partials over each core's own
K/V slice, combined with an in-kernel AllReduce, and patched into the owning
core's output column with copy_predicated.
"""

from collections import deque

import numpy as np
import ml_dtypes

import concourse.bass as bass
import concourse.mybir as mybir
import concourse.tile as tile
from concourse.masks import make_identity
from concourse.bass_utils import run_bass_kernel_spmd

F32 = mybir.dt.float32
F32R = mybir.dt.float32r
BF16 = mybir.dt.bfloat16
F8 = mybir.dt.float8e4
AF = mybir.ActivationFunctionType
ALU = mybir.AluOpType
AX = mybir.AxisListType
DR = mybir.MatmulPerfMode.DoubleRow

D = 1024
FF = 4096
H = 16
HD = 64
T = 2048
B = 2
CHUNK = 512
HALO = 128
NSLOT = 768          # [halo 128 | own 512 | t0 | t2047 | pad]
NKV = 641            # slots 0..640 hold K/V (640 = token0); 641 = q2047 src
NQB = 4
WIN = 256
NEG = -1e30
EPS = 1e-5
N_CORES = 8
WS = 64.0            # fp8 weight pre-scale
SKIP_CC = [False]   # set kernel.SKIP_CC[0]=True to build without the
                    # collective (TimelineSim is single-core only)
PHASE_MARKS = []    # (phase_name, first_inst_id) filled during _emit


def _mark(nc, name):
    PHASE_MARKS.append((name, set(nc.inst_map.keys())))

# ---------------------------------------------------------------- bir fix ---

_waitfix_ctr = [0]


def _split_multiwaits(nc):
    """This container's walrus accepts ONE sync-wait per instruction; Tile
    attaches several. Hoist extras onto NoOps just before each instruction
    (Tile sems are monotonic within a context, so sequential waits are
    equivalent)."""
    n = 0
    for func in nc.m.functions:
        for bb in func.blocks:
            out = []
            changed = False
            for inst in bb.instructions:
                si = inst.sync_info
                if si is not None and len(si.on_wait) > 1:
                    waits = list(si.on_wait)
                    keep = [w for w in waits
                            if getattr(w, "wait_mode", "") not in
                            ("sem-ge-imm", "sem-ge-reg")]
                    if keep:
                        hoist = [w for w in waits if w not in keep]
                        last = keep
                    else:
                        hoist, last = waits[:-1], [waits[-1]]
                    for w in hoist:
                        _waitfix_ctr[0] += 1
                        nop = mybir.InstNoOp(name=f"I-waitfix-{_waitfix_ctr[0]}")
                        nop.engine = inst.engine
                        nop.sync_info = mybir.SyncInfo(on_wait=[w], on_update=[])
                        out.append(nop)
                        n += 1
                    si.on_wait = last
                    changed = True
                out.append(inst)
            if changed:
                bb.instructions[:] = out
    return n

# ------------------------------------------------------------ host helpers --


def _make_x_ext(x, c):
    b, j = divmod(c, 4)
    start = j * CHUNK
    ext = np.zeros((NSLOT, D), np.float32)
    ext[0:HALO] = x[b, start - HALO:start] if j > 0 else x[b, 0:HALO]
    ext[HALO:HALO + CHUNK] = x[b, start:start + CHUNK]
    ext[640] = x[b, 0]
    ext[641] = x[b, T - 1]
    return ext


def _make_mask(c):
    b, j = divmod(c, 4)
    start = j * CHUNK
    m = np.full((NQB, 128, WIN + 1), NEG, np.float32)
    il = np.arange(128)[:, None]
    jl = np.arange(WIN)[None, :]
    for qb in range(NQB):
        q_abs = start + qb * 128 + il
        slot = qb * 128 + jl
        band = (jl >= il) & (jl <= il + 128)
        valid = (j > 0) | (slot >= HALO)
        blk = m[qb, :, :WIN]
        blk[band & valid] = 0.0
        tok0_in_band = (q_abs[:, 0] <= HALO) & (j == 0)
        m[qb, :, WIN] = np.where(tok0_in_band, NEG, 0.0)
    return m


def _tileP(a, p=128):
    """[N*p, ...] -> [p, N, ...] partition-tiled layout."""
    n = a.shape[0] // p
    return np.ascontiguousarray(
        a.reshape(n, p, *a.shape[1:]).transpose(1, 0, *range(2, a.ndim + 1)))


def _vec_t(v):
    return np.ascontiguousarray(np.asarray(v, np.float32).reshape(-1, 128).T)

# ------------------------------------------------------------ bass program --


def _build_nc():
    nc = bass.Bass()

    f8 = mybir.dt
    inp = {}
    for name, shape, dt in [
        ("xT", [128, 8, NSLOT], F32),
        ("wq8", [128, 8, D], F8), ("wk8", [128, 8, D], F8),
        ("wv8", [128, 8, D], F8), ("wo8", [128, 8, D], F8),
        ("w1hi", [128, 8, FF], F8), ("w1lo", [128, 8, FF], F8),
        ("w2hi", [128, 32, D], F8), ("w2lo", [128, 32, D], F8),
        ("msk", [128, NQB, WIN + 1], BF16),
        ("pvec", [128, 6, 8], F32),       # g1,b1,g2,b2,bo,b2
        ("b1h", [128, 32], F32),
        ("fixsel", [128, 1], mybir.dt.uint8),
        ("fAB", [16, 2], F32),
    ]:
        inp[name] = nc.dram_tensor(name, shape, dt, kind="ExternalInput")
    out_d = nc.dram_tensor("outT", [128, 8, CHUNK], F32, kind="ExternalOutput")
    pin = nc.dram_tensor("pin", [H, 2, HD + 1], F32)
    pout = nc.dram_tensor("pout", [H, 2, HD + 1], F32, addr_space="Shared")

    with tile.TileContext(nc) as tc:
        _emit(nc, tc, inp, out_d, pin, pout)
    _split_multiwaits(nc)
    return nc


def _emit(nc, tc, inp, out_d, pin, pout):
    from contextlib import ExitStack
    ctx = ExitStack()
    with ctx:
        pers = ctx.enter_context(tc.tile_pool(name="pers", bufs=1))
        small = ctx.enter_context(tc.tile_pool(name="small", bufs=2))
        bigY = ctx.enter_context(tc.tile_pool(name="bigY", bufs=1))
        big0 = ctx.enter_context(tc.tile_pool(name="big0", bufs=1))
        w1p = ctx.enter_context(tc.tile_pool(name="w1p", bufs=3))

        # ---- persistent constants / params
        pvec = pers.tile([128, 6, 8], F32, tag="pvec", name="pvec")
        b1h = pers.tile([128, 32], F32, tag="b1h", name="b1h")
        msk = pers.tile([128, NQB, WIN + 1], BF16, tag="msk")
        fixsel = pers.tile([128, 1], mybir.dt.uint8, tag="fixsel")
        fAB = pers.tile([16, 2], F32, tag="fAB")

        idf = pers.tile([128, 128], F32, tag="idf")
        idb = pers.tile([128, 128], BF16, tag="idb")
        onesp = pers.tile([128, 1], BF16, tag="onesp")
        epst = pers.tile([1, 1], F32, tag="epst")
        neg3 = pers.tile([128, 1], F32, tag="neg3")

        yT = bigY.tile([128, 8, CHUNK], F32, tag="yT")
        h2hi = bigY.tile([128, 8, CHUNK], F8, tag="h2hi")
        h2lo = bigY.tile([128, 8, CHUNK], F8, tag="h2lo")
        w2hi = big0.tile([128, 32, D], F8, tag="w2hi")
        w1sl = {}   # FFN1 weight slab stream: g -> (hi, lo), 4 m-tiles each

        def w1_issue(g):
            if g >= 8:
                return
            hi = w1p.tile([128, 8, 512], F8, tag="w1hi")
            lo = w1p.tile([128, 8, 512], F8, tag="w1lo")
            gs = slice(g * 512, (g + 1) * 512)
            nc.sync.dma_start(out=hi, in_=inp["w1hi"][:, :, gs])
            nc.sync.dma_start(out=lo, in_=inp["w1lo"][:, :, gs])
            w1sl[g] = (hi, lo)

        def g_(i, kt):   # per-partition scalar views of packed params
            return pvec[:, i, kt:kt + 1]

        # ================= LN in transposed layout (bf16 stats) ============
        def layernorm_T(src, chunks, emit_out, pools):
            """emit_out(kt, sl, t2): consume normalized bf16 (pre-g/b).
            chunks: list of (start, end) column ranges to process."""
            ps_row, ps_bc, scr, rowp = pools
            width = max(e for _, e in chunks)
            mus = []
            for (cs, ce) in chunks:
                mus.append((ps_row.tile([1, ce - cs], F32, tag="row",
                                        name="mu"),
                            ps_row.tile([1, ce - cs], F32, tag="row",
                                        name="msq")))
            onesb = scr.tile([128, 1], BF16, tag="ln_onesb")
            nc.vector.memset(onesb, 1.0 / D)
            ones1b = scr.tile([1, 128], BF16, tag="ln_ones1b")
            nc.vector.memset(ones1b, 1.0)
            for kt in range(8):
                xb = scr.tile([128, width], BF16, tag="ln_xb")
                nc.scalar.copy(out=xb, in_=src[:, kt, 0:width])
                xsq = scr.tile([128, width], BF16, tag="ln_xsq")
                nc.vector.tensor_mul(out=xsq, in0=xb, in1=xb)
                for cch, (cs, ce) in enumerate(chunks):
                    sl = slice(cs, ce)
                    nc.tensor.matmul(mus[cch][0], onesb, xb[:, sl],
                                     start=kt == 0, stop=kt == 7)
                    nc.tensor.matmul(mus[cch][1], onesb, xsq[:, sl],
                                     start=kt == 0, stop=kt == 7)
            bcs = []
            for cch, (cs, ce) in enumerate(chunks):
                cw = ce - cs
                mu_ps, msq_ps = mus[cch]
                musb = rowp.tile([1, cw], F32, tag="ln_mu")
                nc.scalar.copy(out=musb, in_=mu_ps)
                tmp = rowp.tile([1, cw], F32, tag="ln_tmp")
                nc.vector.tensor_mul(out=tmp, in0=musb, in1=musb)
                nc.vector.tensor_sub(out=tmp, in0=msq_ps, in1=tmp)
                nc.scalar.activation(out=tmp, in_=tmp, func=AF.Sqrt,
                                     bias=epst, scale=1.0)
                nc.vector.reciprocal(out=tmp, in_=tmp)       # rstd
                tmpb = rowp.tile([1, cw], BF16, tag="ln_tmpb")
                nc.vector.tensor_copy(out=tmpb, in_=tmp)
                nc.vector.tensor_mul(out=musb, in0=musb, in1=tmp)
                musbb = rowp.tile([1, cw], BF16, tag="ln_musbb")
                nc.scalar.mul(out=musbb, in_=musb, mul=-1.0)  # -mu*rstd
                rb_ps = ps_bc.tile([128, cw], F32, tag="bc", name="rb")
                nc.tensor.matmul(rb_ps, ones1b, tmpb, start=True, stop=True)
                nb_ps = ps_bc.tile([128, cw], F32, tag="bc", name="nb")
                nc.tensor.matmul(nb_ps, ones1b, musbb, start=True, stop=True)
                rb_sb = scr.tile([128, cw], BF16, tag="ln_rb")
                nc.scalar.copy(out=rb_sb, in_=rb_ps)
                nb_sb = scr.tile([128, cw], BF16, tag="ln_nb")
                nc.scalar.copy(out=nb_sb, in_=nb_ps)
                bcs.append((rb_sb, nb_sb))
            for kt in range(8):
                for cch, (cs, ce) in enumerate(chunks):
                    sl = slice(cs, ce)
                    rb_sb, nb_sb = bcs[cch]
                    t1 = scr.tile([128, ce - cs], BF16, tag="ln_t1")
                    nc.vector.tensor_mul(out=t1, in0=src[:, kt, sl],
                                         in1=rb_sb)
                    t2 = scr.tile([128, ce - cs], BF16, tag="ln_t2")
                    nc.vector.tensor_add(out=t2, in0=t1, in1=nb_sb)
                    emit_out(kt, sl, t2)

        with tc.tile_pool(name="bigA", bufs=1) as bigA:
            # ---- long-lived activations (until end of phase G)
            xT = bigA.tile([128, 8, NSLOT], F32, tag="xT")
            nc.sync.dma_start(out=xT[:, 0, 0:384], in_=inp["xT"][:, 0, 0:384])
            nc.sync.dma_start(out=xT[:, 0, 384:NSLOT],
                              in_=inp["xT"][:, 0, 384:NSLOT])
            for kt in range(1, 8):
                nc.sync.dma_start(out=xT[:, kt, :], in_=inp["xT"][:, kt, :])
            nc.sync.dma_start(out=pvec, in_=inp["pvec"][:])
            nc.sync.dma_start(out=b1h, in_=inp["b1h"][:])
            nc.sync.dma_start(out=msk, in_=inp["msk"][:])
            nc.sync.dma_start(out=fixsel, in_=inp["fixsel"][:])
            nc.sync.dma_start(out=fAB, in_=inp["fAB"][:])
            make_identity(nc, idf)
            make_identity(nc, idb)
            nc.vector.memset(onesp, 1.0)
            nc.vector.memset(epst, EPS)
            nc.vector.memset(neg3, -3.0)

            wo8 = bigA.tile([128, 8, D], F8, tag="wo8")
            h8 = bigA.tile([128, 8, NSLOT], F8, tag="h8")
            QT = bigA.tile([128, 8, CHUNK], BF16, tag="QT")
            q47T = bigA.tile([128, 8], BF16, tag="q47T")
            KT = bigA.tile([128, 8, NKV], BF16, tag="KT")
            V = bigA.tile([128, 6, D], BF16, tag="V")
            OT = bigA.tile([128, 8, CHUNK], F8, tag="OT")

            with tc.tile_pool(name="bigW", bufs=1) as bigW:
                wq8 = bigW.tile([128, 8, D], F8, tag="wq8")
                nc.sync.dma_start(out=wq8[:, :, 0:512],
                                  in_=inp["wq8"][:, :, 0:512])
                wk8 = bigW.tile([128, 8, D], F8, tag="wk8")
                nc.sync.dma_start(out=wk8[:, :, 0:512],
                                  in_=inp["wk8"][:, :, 0:512])
                nc.sync.dma_start(out=wq8[:, :, 512:D],
                                  in_=inp["wq8"][:, :, 512:D])
                nc.sync.dma_start(out=wk8[:, :, 512:D],
                                  in_=inp["wk8"][:, :, 512:D])
                wv8 = bigW.tile([128, 8, D], F8, tag="wv8")
                nc.sync.dma_start(out=wv8, in_=inp["wv8"][:])
                nc.sync.dma_start(out=wo8, in_=inp["wo8"][:])
                for g in range(4):
                    gs = slice(g * 8, (g + 1) * 8)
                    nc.sync.dma_start(out=w2hi[:, gs, :],
                                      in_=inp["w2hi"][:, gs, :])

                _mark(nc, "B:ln1")
                # ============= Phase B: LN1 -> h8 (fp8) ====================
                with tc.tile_pool(name="ps_row1", bufs=4, space="PSUM") as ps_row, \
                     tc.tile_pool(name="ps_bc1", bufs=4, space="PSUM") as ps_bc, \
                     tc.tile_pool(name="lnrow1", bufs=1) as lnrow, \
                     tc.tile_pool(name="lnscr1", bufs=2) as lnscr:
                    def ln1_out(kt, sl, t2):
                        nc.gpsimd.tensor_scalar(
                            out=h8[:, kt, sl], in0=t2,
                            scalar1=g_(0, kt), scalar2=g_(1, kt),
                            op0=ALU.mult, op1=ALU.add)
                    layernorm_T(xT, [(0, 384), (384, 642)], ln1_out,
                                (ps_row, ps_bc, lnscr, lnrow))

                _mark(nc, "C:qkv")
                # ============= Phase C: QKV (fp8 DoubleRow) ================
                with tc.tile_pool(name="ps_big", bufs=6, space="PSUM") as ps_big, \
                     tc.tile_pool(name="ps_tiny", bufs=2, space="PSUM") as ps_tiny:
                    for mp in range(4):
                        ms = [2 * mp, 2 * mp + 1]
                        qp, kp, k2p, q47p = {}, {}, {}, {}
                        for m in ms:
                            qp[m] = ps_big.tile([128, CHUNK], F32,
                                                tag="big", name="q_ps")
                            kp[m] = ps_big.tile([128, 512], F32,
                                                tag="big", name="k_ps")
                            k2p[m] = ps_big.tile([128, NKV - 512], F32,
                                                 tag="big", name="k2_ps")
                            q47p[m] = ps_tiny.tile([128, 1], F32, tag="tiny",
                                                   name="q47_ps")
                        for t in range(0, 8, 2):
                            for m in ms:
                                msl = slice(m * 128, (m + 1) * 128)
                                nc.tensor.matmul(
                                    qp[m], wq8[:, t:t + 2, msl],
                                    h8[:, t:t + 2, HALO:HALO + CHUNK],
                                    start=t == 0, stop=t == 6, perf_mode=DR)
                                nc.tensor.matmul(
                                    kp[m], wk8[:, t:t + 2, msl],
                                    h8[:, t:t + 2, 0:512],
                                    start=t == 0, stop=t == 6, perf_mode=DR)
                                nc.tensor.matmul(
                                    k2p[m], wk8[:, t:t + 2, msl],
                                    h8[:, t:t + 2, 512:NKV],
                                    start=t == 0, stop=t == 6, perf_mode=DR)
                                nc.tensor.matmul(
                                    q47p[m], wq8[:, t, msl],
                                    h8[:, t, 641:642],
                                    start=t == 0, stop=False)
                                nc.tensor.matmul(
                                    q47p[m], wq8[:, t + 1, msl],
                                    h8[:, t + 1, 641:642],
                                    start=False, stop=t == 6)
                        for m in ms:
                            nc.vector.tensor_scalar_mul(
                                out=QT[:, m, :], in0=qp[m],
                                scalar1=1.0 / (WS * np.sqrt(HD)))
                            nc.scalar.mul(out=q47T[:, m:m + 1], in_=q47p[m],
                                          mul=1.0 / (WS * np.sqrt(HD)))
                            nc.scalar.mul(out=KT[:, m, 0:512], in_=kp[m],
                                          mul=1.0 / WS)
                            nc.vector.tensor_scalar_mul(
                                out=KT[:, m, 512:NKV], in0=k2p[m],
                                scalar1=1.0 / WS)
                    for tt in range(6):
                        for cch in range(2):
                            csl = slice(cch * 512, (cch + 1) * 512)
                            v_ps = ps_big.tile([128, 512], F32, tag="big")
                            for t in range(0, 8, 2):
                                nc.tensor.matmul(
                                    v_ps,
                                    h8[:, t:t + 2, tt * 128:(tt + 1) * 128],
                                    wv8[:, t:t + 2, csl],
                                    start=t == 0, stop=t == 6, perf_mode=DR)
                            nc.scalar.mul(out=V[:, tt, csl][:, 0:256],
                                          in_=v_ps[:, 0:256], mul=1.0 / WS)
                            nc.vector.tensor_scalar_mul(
                                out=V[:, tt, csl][:, 256:512],
                                in0=v_ps[:, 256:512], scalar1=1.0 / WS)

                    _mark(nc, "D:partials")
                    # ========= Phase D: global-row partials + AllReduce ====
                    s47_ps = ps_tiny.tile([128, H * 4], F32, tag="tiny",
                                          name="s47_ps")
                    for h in range(H):
                        p0 = 64 * (h % 2)
                        for i in range(4):
                            nc.tensor.matmul(
                                s47_ps[:, 4 * h + i:4 * h + i + 1],
                                KT[p0:p0 + 64, h // 2,
                                   HALO + 128 * i:HALO + 128 * (i + 1)],
                                q47T[p0:p0 + 64, h // 2:h // 2 + 1],
                                start=True, stop=True)
                    p47 = small.tile([128, H * 4], BF16, tag="p_p47")
                    nc.scalar.activation(out=p47, in_=s47_ps, func=AF.Exp)
                    ssum_ps = ps_tiny.tile([1, H * 4], F32, tag="tiny")
                    nc.tensor.matmul(ssum_ps, onesp, p47, start=True,
                                     stop=True)
                    s_c = small.tile([1, H], F32, tag="p_sc")
                    nc.vector.reduce_sum(
                        out=s_c,
                        in_=ssum_ps.rearrange("p (h i) -> p h i", i=4),
                        axis=AX.X)
                    oall = small.tile([65, H], F32, tag="p_oall")
                    o47_ps = ps_tiny.tile([64, H], F32, tag="tiny",
                                          name="o47_ps")
                    for h in range(H):
                        for i in range(4):
                            nc.tensor.matmul(
                                o47_ps[:, h:h + 1],
                                V[:, 1 + i, 64 * h:64 * h + 64],
                                p47[:, 4 * h + i:4 * h + i + 1],
                                start=i == 0, stop=i == 3)
                    nc.scalar.copy(out=oall[0:64, :], in_=o47_ps)
                    nc.sync.dma_start(out=oall[64:65, :], in_=s_c)
                    part_ps = ps_tiny.tile([H, 65], F32, tag="tiny")
                    nc.tensor.transpose(part_ps, oall, idf[0:65, 0:65])
                    part_sb = small.tile([H, 65], F32, tag="p_part")
                    nc.scalar.copy(out=part_sb, in_=part_ps)
                    pa = small.tile([H, 2, 65], F32, tag="p_pa")
                    nc.vector.tensor_scalar_mul(out=pa[:, 0, :], in0=part_sb,
                                                scalar1=fAB[:, 0:1])
                    nc.vector.tensor_scalar_mul(out=pa[:, 1, :], in0=part_sb,
                                                scalar1=fAB[:, 1:2])
                    nc.sync.dma_start(out=pin[:], in_=pa)
                    if not SKIP_CC[0]:
                        nc.gpsimd.collective_compute(
                            "AllReduce", ALU.add,
                            replica_groups=[[0, 1, 2, 3, 4, 5, 6, 7]],
                            ins=[pin[:]], outs=[pout[:]])
            for g in range(3):
                w1_issue(g)

            _mark(nc, "E:attn")
            # ============= Phase E: windowed attention =====================
            with tc.tile_pool(name="ps_s", bufs=2, space="PSUM") as ps_s, \
                 tc.tile_pool(name="ps_pt", bufs=2, space="PSUM") as ps_pt, \
                 tc.tile_pool(name="ps_p0", bufs=1, space="PSUM") as ps_p0, \
                 tc.tile_pool(name="ps_o", bufs=1, space="PSUM") as ps_o, \
                 tc.tile_pool(name="ascr", bufs=3) as ascr:
                opr = [ps_o.tile([128, NQB, 128], F32, tag="o", name="o")]

                def attn_front(u):
                    pr, qb = u
                    s2 = ps_s.tile([128, 2, 512], F32, tag="s2")
                    for sub in range(2):
                        p0 = 64 * sub
                        qs = QT[p0:p0 + 64, pr, qb * 128:(qb + 1) * 128]
                        nc.tensor.matmul(
                            s2[:, sub, 0:WIN], qs,
                            KT[p0:p0 + 64, pr, qb * 128:qb * 128 + WIN],
                            start=True, stop=False)
                        nc.tensor.matmul(s2[:, sub, WIN:WIN + 1], qs,
                                         KT[p0:p0 + 64, pr, 640:641],
                                         start=False, stop=False)
                        nc.tensor.matmul(s2[:, sub, 0:WIN + 1], idb,
                                         msk[:, qb, :],
                                         start=False, stop=True)
                    return s2

                def attn_back(u, s2):
                    pr, qb = u
                    p1 = ascr.tile([128, 2, WIN + 1], BF16, tag="a_p1")
                    nc.scalar.activation(out=p1, in_=s2[:, :, 0:WIN + 1],
                                         func=AF.Exp, bias=neg3, scale=1.0)
                    rs = ascr.tile([128, 2], F32, tag="a_rs")
                    nc.vector.reduce_sum(out=rs, in_=p1, axis=AX.X)
                    rc = ascr.tile([128, 2], F32, tag="a_rc")
                    nc.vector.reciprocal(out=rc, in_=rs)
                    p8 = ascr.tile([128, 2, WIN + 1], BF16, tag="a_p8")
                    for sub in range(2):
                        nc.gpsimd.tensor_scalar_mul(
                            out=p8[:, sub], in0=p1[:, sub],
                            scalar1=rc[:, sub:sub + 1])
                    pt = ps_pt.tile([128, 2, 2, 128], BF16, tag="pt")
                    pt0 = ps_p0.tile([1, 2, 128], BF16, tag="pt0")
                    for sub in range(2):
                        nc.tensor.transpose(pt[:, sub, 0, :],
                                            p8[:, sub, 0:128], idb)
                        nc.tensor.transpose(pt[:, sub, 1, :],
                                            p8[:, sub, 128:256], idb)
                        nc.tensor.transpose(pt0[:, sub, :],
                                            p8[:, sub, WIN:WIN + 1], idb)
                    ptb = ascr.tile([128, 2, 2, 128], BF16, tag="a_ptb")
                    if qb % 2 == 0:
                        nc.vector.tensor_copy(out=ptb, in_=pt)
                    else:
                        nc.scalar.copy(out=ptb, in_=pt)
                    pt0b = ascr.tile([1, 2, 128], BF16, tag="a_pt0b")
                    if qb % 2 == 0:
                        nc.scalar.copy(out=pt0b, in_=pt0)
                    else:
                        nc.vector.tensor_copy(out=pt0b, in_=pt0)
                    o_ps = opr[0]
                    for sub in range(2):
                        p0 = 64 * sub
                        dv = slice(64 * (2 * pr + sub),
                                   64 * (2 * pr + sub) + 64)
                        nc.tensor.matmul(o_ps[p0:p0 + 64, qb, :],
                                         V[:, qb, dv], ptb[:, sub, 0, :],
                                         start=True, stop=False)
                        nc.tensor.matmul(o_ps[p0:p0 + 64, qb, :],
                                         V[:, qb + 1, dv], ptb[:, sub, 1, :],
                                         start=False, stop=False)
                        nc.tensor.matmul(o_ps[p0:p0 + 64, qb, :],
                                         V[0:1, 5, dv],
                                         pt0b[:, sub, :], start=False,
                                         stop=True)
                    if qb == NQB - 1:
                        if pr % 2 == 0:
                            nc.vector.tensor_copy(out=OT[:, pr, :],
                                                  in_=o_ps.rearrange(
                                                      "p a b -> p (a b)"))
                        else:
                            nc.scalar.copy(out=OT[:, pr, :],
                                           in_=o_ps.rearrange(
                                               "p a b -> p (a b)"))
                        opr[0] = ps_o.tile([128, NQB, 128], F32, tag="o", name="o")

                units = [(pr, qb) for pr in range(8) for qb in range(NQB)]
                pend = deque()
                for u in units:
                    pend.append((u, attn_front(u)))
                    if len(pend) >= 2:
                        attn_back(*pend.popleft())
                while pend:
                    attn_back(*pend.popleft())

            _mark(nc, "F:patch")
            # ============= Phase F: gather + patch global row ==============
            with tc.tile_pool(name="ps_tf", bufs=2, space="PSUM") as ps_tf:
                gath = small.tile([H, 2, 65], F32, tag="p_gath")
                nc.sync.dma_start(out=gath,
                                  in_=(pin if SKIP_CC[0] else pout)[:])
                vA = small.tile([H, 65], F32, tag="p_vA")
                nc.vector.tensor_scalar_mul(out=vA, in0=gath[:, 0, :],
                                            scalar1=fAB[:, 0:1])
                vB = small.tile([H, 65], F32, tag="p_vB")
                nc.vector.tensor_scalar_mul(out=vB, in0=gath[:, 1, :],
                                            scalar1=fAB[:, 1:2])
                val = small.tile([H, 65], F32, tag="p_val")
                nc.vector.tensor_add(out=val, in0=vA, in1=vB)
                recS = small.tile([H, 1], F32, tag="p_recS")
                nc.vector.reciprocal(out=recS, in_=val[:, 64:65])
                a47 = small.tile([H, HD], F32, tag="p_a47")
                nc.vector.tensor_scalar_mul(out=a47, in0=val[:, 0:64],
                                            scalar1=recS)
                a47t_ps = ps_tf.tile([HD, H], F32, tag="tf")
                nc.tensor.transpose(a47t_ps, a47, idf[0:H, 0:H])
                a47T = small.tile([HD, H], F8, tag="p_a47T")
                nc.scalar.copy(out=a47T, in_=a47t_ps)
                fix_sb = small.tile([128, 8], F8, tag="p_fix")
                a47v = a47T.rearrange("p (t two) -> p t two", two=2)
                nc.sync.dma_start(out=fix_sb[0:64, :], in_=a47v[:, :, 0])
                nc.sync.dma_start(out=fix_sb[64:128, :], in_=a47v[:, :, 1])
                for t in range(8):
                    nc.vector.copy_predicated(out=OT[:, t, CHUNK - 1:CHUNK],
                                              mask=fixsel,
                                              data=fix_sb[:, t:t + 1])

            _mark(nc, "G:wo")
            # ============= Phase G: out-proj (fp8 DR) + residual ===========
            with tc.tile_pool(name="ps_g", bufs=4, space="PSUM") as ps_g, \
                 tc.tile_pool(name="gscr", bufs=2) as gscr:
                for m in range(8):
                    msl = slice(m * 128, (m + 1) * 128)
                    pr_ps = ps_g.tile([128, CHUNK], F32, tag="g")
                    for t in range(0, 8, 2):
                        nc.tensor.matmul(pr_ps, wo8[:, t:t + 2, msl],
                                         OT[:, t:t + 2, :],
                                         start=t == 0, stop=t == 6,
                                         perf_mode=DR)
                    y1 = gscr.tile([128, CHUNK], F32, tag="evac512")
                    nc.scalar.activation(out=y1, in_=pr_ps, func=AF.Identity,
                                         bias=g_(4, m), scale=1.0 / WS)
                    nc.vector.tensor_add(out=yT[:, m, :], in0=y1,
                                         in1=xT[:, m, HALO:HALO + CHUNK])

        # ---- bigA closed: attention-era SBUF freed
        with tc.tile_pool(name="bigF", bufs=1) as bigF:
            ht = bigF.tile([128, 32, CHUNK], F8, tag="ht")
            w2lo = bigF.tile([128, 32, D], F8, tag="w2lo")

            _mark(nc, "H:ln2")
            # ============= Phase H: LN2 -> h2hi/h2lo (fp8) =================
            with tc.tile_pool(name="ps_row2", bufs=2, space="PSUM") as ps_row2, \
                 tc.tile_pool(name="ps_bc2", bufs=2, space="PSUM") as ps_bc2, \
                 tc.tile_pool(name="lnrow2", bufs=1) as lnrow2, \
                 tc.tile_pool(name="lnscr2", bufs=2) as lnscr2:
                def ln2_out(kt, sl, t2):
                    h2b = lnscr2.tile([128, CHUNK], BF16, tag="ln2_h2b")
                    nc.gpsimd.tensor_scalar(
                        out=h2b, in0=t2,
                        scalar1=g_(2, kt), scalar2=g_(3, kt),
                        op0=ALU.mult, op1=ALU.add)
                    nc.scalar.copy(out=h2hi[:, kt, :], in_=h2b)
                    nc.vector.tensor_sub(out=h2lo[:, kt, :], in0=h2b,
                                         in1=h2hi[:, kt, :])
                layernorm_T(yT, [(0, CHUNK)], ln2_out,
                            (ps_row2, ps_bc2, lnscr2, lnrow2))

            _mark(nc, "I:ffn1")
            # ============= Phase I: FFN1 (compensated fp8) -> ht ===========
            with tc.tile_pool(name="ps_f1", bufs=4, space="PSUM") as ps_f1:
                for g in range(8):
                    w1_issue(g + 3)
                    if g == 5:
                        for wg in range(4):
                            wgs = slice(wg * 8, (wg + 1) * 8)
                            nc.sync.dma_start(out=w2lo[:, wgs, :],
                                              in_=inp["w2lo"][:, wgs, :])
                    whi, wlo = w1sl[g]
                    for ml in range(4):
                        m = 4 * g + ml
                        msl = slice(ml * 128, (ml + 1) * 128)
                        h_ps = ps_f1.tile([128, CHUNK], F32, tag="f1")
                        for t in range(0, 8, 2):
                            nc.tensor.matmul(h_ps, whi[:, t:t + 2, msl],
                                             h2hi[:, t:t + 2, :],
                                             start=t == 0, stop=False,
                                             perf_mode=DR)
                            nc.tensor.matmul(h_ps, whi[:, t:t + 2, msl],
                                             h2lo[:, t:t + 2, :],
                                             start=False, stop=False,
                                             perf_mode=DR)
                            nc.tensor.matmul(h_ps, wlo[:, t:t + 2, msl],
                                             h2hi[:, t:t + 2, :],
                                             start=False, stop=t == 6,
                                             perf_mode=DR)
                        nc.scalar.activation(out=ht[:, m, :], in_=h_ps,
                                             func=AF.Gelu,
                                             bias=b1h[:, m:m + 1],
                                             scale=1.0 / WS)

            _mark(nc, "J:ffn2")
            # ============= Phase J: FFN2 (W-comp fp8) + residual + out =====
            with tc.tile_pool(name="ps_f2", bufs=4, space="PSUM") as ps_f2, \
                 tc.tile_pool(name="jscr", bufs=2) as jscr:
                for mo in range(8):
                    msl = slice(mo * 128, (mo + 1) * 128)
                    halves = [(0, CHUNK)] if mo < 7 else [(0, 256),
                                                          (256, CHUNK)]
                    for (c0, c1) in halves:
                        f2_ps = ps_f2.tile([128, c1 - c0], F32, tag="f2")
                        for t in range(0, 32, 2):
                            nc.tensor.matmul(f2_ps, w2hi[:, t:t + 2, msl],
                                             ht[:, t:t + 2, c0:c1],
                                             start=t == 0, stop=False,
                                             perf_mode=DR)
                            nc.tensor.matmul(f2_ps, w2lo[:, t:t + 2, msl],
                                             ht[:, t:t + 2, c0:c1],
                                             start=False, stop=t == 30,
                                             perf_mode=DR)
                        f1 = jscr.tile([128, c1 - c0], F32, tag="evac512")
                        nc.scalar.activation(out=f1, in_=f2_ps,
                                             func=AF.Identity,
                                             bias=g_(5, mo), scale=1.0 / WS)
                        om = jscr.tile([128, c1 - c0], F32, tag="out_m")
                        nc.vector.tensor_add(out=om, in0=f1,
                                             in1=yT[:, mo, c0:c1])
                        nc.sync.dma_start(out=out_d[:, mo, c0:c1], in_=om)

# ------------------------------------------------------------------ driver --

_CACHE = {}


def _prep_core_inputs(inputs, c, shared_cache={}):
    bf = ml_dtypes.bfloat16
    f8 = ml_dtypes.float8_e4m3

    def q8(a):
        return np.ascontiguousarray(a).astype(f8)

    key = id(inputs.get("Wq"))
    shared = shared_cache.get(key)
    if shared is None:
        shared_cache.clear()
        w1t = _tileP(np.asarray(inputs["W1"], np.float32)) * WS  # [128,8,FF]
        w1hi = q8(w1t)
        w1lo = q8(w1t - w1hi.astype(np.float32))
        w2t = np.ascontiguousarray(
            np.asarray(inputs["W2"], np.float32)
            .reshape(32, 128, D).transpose(1, 0, 2)) * WS        # [128,32,D]
        w2hi = q8(w2t)
        w2lo = q8(w2t - w2hi.astype(np.float32))
        pv = np.stack([_vec_t(inputs[k]) for k in
                       ("ln1_g", "ln1_b", "ln2_g", "ln2_b", "bo", "b2")],
                      axis=1)                                     # [128,6,8]
        shared = {
            "wq8": q8(_tileP(np.asarray(inputs["Wq"], np.float32)) * WS),
            "wk8": q8(_tileP(np.asarray(inputs["Wk"], np.float32)) * WS),
            "wv8": q8(_tileP(np.asarray(inputs["Wv"], np.float32)) * WS),
            "wo8": q8(_tileP(np.asarray(inputs["Wo"], np.float32)) * WS),
            "w1hi": w1hi, "w1lo": w1lo, "w2hi": w2hi, "w2lo": w2lo,
            "pvec": np.ascontiguousarray(pv, dtype=np.float32),
            "b1h": np.ascontiguousarray(
                np.asarray(inputs["b1"], np.float32).reshape(32, 128).T),
        }
        shared_cache[key] = shared
    x = np.asarray(inputs["x"], np.float32)
    xT = np.ascontiguousarray(
        _make_x_ext(x, c).T.reshape(8, 128, NSLOT).transpose(1, 0, 2))
    msk = np.ascontiguousarray(
        _make_mask(c).transpose(1, 0, 2)).astype(bf)
    fs = np.full((128, 1), 1 if c % 4 == 3 else 0, np.uint8)
    fAB = np.zeros((16, 2), np.float32)
    fAB[:, 0] = 1.0 if c < 4 else 0.0
    fAB[:, 1] = 0.0 if c < 4 else 1.0
    return {**shared, "xT": xT, "msk": msk, "fixsel": fs, "fAB": fAB}


def get_nc():
    if "nc" not in _CACHE:
        _CACHE["nc"] = _build_nc()
    return _CACHE["nc"]


def kernel(**inputs):
    nc = get_nc()
    in_maps = [_prep_core_inputs(inputs, c) for c in range(N_CORES)]
    res = run_bass_kernel_spmd(nc, in_maps, core_ids=list(range(N_CORES)),
                               trace=False)
    out = np.zeros((B, T, D), np.float32)
    for c in range(N_CORES):
        b, j = divmod(c, 4)
        oT = res.results[c]["outT"]          # [128, 8, 512]
        out[b, j * CHUNK:(j + 1) * CHUNK] = \
            oT.transpose(1, 0, 2).reshape(D, CHUNK).T
    return out
